# revision 1
# baseline (speedup 1.0000x reference)
"""Trainium2 Bass kernel for nn_NewAttention (analytic Gaussian sparse attention).

Math (per batch element b):
    v        = x[b] @ W_in.T                      # [L, E]
    per head h (P=128 cols of v):
        A_h  = softmax(-(j - c_h(i))^2 / 2)       # [L, L], analytic, banded
        att_h = A_h @ v_h                         # [L, P]
    out[b]   = concat_h(att_h) @ W_out.T          # [L, E]

Sharding: data-parallel over batch, one batch element per NeuronCore (8 cores).

Device strategy (per core):
  - host pre-transposes x[b] -> xT [E, L] so matmul1 needs no on-chip transpose
  - matmul1: v[l-tile, m] = xT-slice.T @ W_inT (stationary = xT 128x128 slices,
    moving = W_inT 512-chunks) -> v in natural layout, resident in SBUF.
  - attention: att^T_h = v_h.T @ A_h^T computed as banded matmuls: stationary =
    v 128x128 slices (contraction over sequence), moving = analytic A^T blocks
    [128, 256] (host-precomputed exact softmax weights; only 7 distinct blocks
    per head type thanks to shift invariance). Each v-tile's band covers a
    q-window padded to N=256 (full-rate float32r) and PSUM accumulates
    overlapping windows via the per-element has_written bits; the first matmul
    touching each PSUM bank uses start=True (whole-bank has_written clear).
    Output is feature-major att^T, exactly what matmul2 needs as stationary.
  - 'first'/'last' heads attend to a fixed key location for every query, so
    their output contribution is a rank-1 update r[e] = (w_h @ v_h) @ W_outT_h,
    broadcast across partitions once (K=1 matmul) and added by the DVE during
    the PSUM->SBUF copy of each output tile. Their v columns are only needed
    at the sequence boundary, so matmul1 skips them for interior tiles.
  - matmul2: out[l-tile, e] = att^T-slice.T @ W_outT chunks -> natural layout,
    contiguous DMA out.

All matmuls run in float32r (TF32-like full-rate fp32: 1 cyc/row at moving
dim >= 256). Measured end-to-end relative error ~2.3e-4.
"""

import sys
import numpy as np

for _p in ("/opt/trn_rl_repo",):
    if _p not in sys.path:
        sys.path.insert(0, _p)

import concourse.bass as bass
import concourse.bacc as bacc
import concourse.mybir as mybir
from concourse import tile
from concourse import bass2jax as _b2j

# ---------------- problem constants (hardcoded per contract) ----------------
B = 8
L = 2048
E = 1024
H = 8
P = 128
SIGMA = 1.0
DISP = 1
NT = L // 128          # 16 l-tiles
KT = E // 128          # 8 k-tiles
NSLAB = 2              # attention q-slabs of 1024
DT = mybir.dt.float32
MM_DT = mybir.dt.float32r

BANDED_HEADS = [0, 1, 2, 5, 6, 7]   # center,left,right,center,left,right
NBH = len(BANDED_HEADS)
HEAD_TYPE = {0: 0, 1: 1, 2: 2, 5: 0, 6: 1, 7: 2}  # 0=center,1=left,2=right
TYPE_DISP = [0, -DISP, +DISP]

# attention A^T block classes: (name, representative tile i0, start_rel)
# window for tile i, class c = [128*i + start_rel, 128*i + start_rel + 256)
CLS = [
    ("int8", 1, -8),      # interior single-window tiles (i%4 in {1,2})
    ("intA", 3, -128),    # i%4==3 piece A
    ("intB", 3, +128),    # i%4==3 piece B
    ("prevA", 4, -256),   # i%4==0 (i>0) piece A
    ("int0", 4, 0),       # i%4==0 (i>0) piece B
    ("first0", 0, 0),     # i==0 (boundary-renormalized rows)
    ("last", 15, -128),   # i==15 (boundary-renormalized rows)
]
CLS_IDX = {name: k for k, (name, _, _) in enumerate(CLS)}
NCLS = len(CLS)


def _pieces(i):
    """A^T matmul pieces for v-tile i: list of (start_rel, class_idx)."""
    if i == 0:
        return [(0, CLS_IDX["first0"])]
    if i == NT - 1:
        return [(-128, CLS_IDX["last"])]
    m = i % 4
    if m in (1, 2):
        return [(-8, CLS_IDX["int8"])]
    if m == 3:
        return [(-128, CLS_IDX["intA"]), (+128, CLS_IDX["intB"])]
    return [(-256, CLS_IDX["prevA"]), (0, CLS_IDX["int0"])]


def _softmax_rows(logits):
    m = logits.max(axis=-1, keepdims=True)
    e = np.exp(logits - m)
    return e / e.sum(axis=-1, keepdims=True)


def _host_tables():
    """Analytic attention weight blocks (exact, float64 -> fp32).

    a_all[p, (t*NCLS + c)*256 + q] = A_t[q0 + q, 128*i0 + p]
    where (i0, start_rel) come from CLS[c] and q0 = 128*i0 + start_rel
    (rows outside [0, L) are zero; none occur by construction).
    """
    j = np.arange(L, dtype=np.float64)
    i = np.arange(L, dtype=np.float64)

    a_all = np.zeros((128, 3 * NCLS * 256), dtype=np.float64)
    for t, disp in enumerate(TYPE_DISP):
        c = i + disp
        logits = -((j[None, :] - c[:, None]) ** 2) / (2.0 * SIGMA**2)
        A = _softmax_rows(logits)  # [Lq, Lk]
        for ci, (_, i0, start_rel) in enumerate(CLS):
            q0 = 128 * i0 + start_rel
            assert 0 <= q0 and q0 + 256 <= L, (i0, start_rel)
            blkcol = (t * NCLS + ci) * 256
            a_all[:, blkcol : blkcol + 256] = A[
                q0 : q0 + 256, 128 * i0 : 128 * i0 + 128
            ].T

    # first/last heads: fixed weight vector over keys (same for every query)
    Af = _softmax_rows(-((j[None, :] - np.zeros((1, 1))) ** 2) / (2 * SIGMA**2))
    Al = _softmax_rows(
        -((j[None, :] - np.full((1, 1), L - 1.0)) ** 2) / (2 * SIGMA**2)
    )
    wfl = np.zeros((128, 2), dtype=np.float64)
    wfl[:, 0] = Af[0, 0:128]         # support at k < 128  (v tile 0)
    wfl[:, 1] = Al[0, L - 128 : L]   # support at k >= L-128 (v tile 15)

    return a_all.astype(np.float32), wfl.astype(np.float32)


def _build_program(phases=3):
    nc = bacc.Bacc("TRN2", target_bir_lowering=False, debug=False, num_devices=B)

    xT = nc.dram_tensor("xT", [E, L], MM_DT, kind="ExternalInput")
    w_inT = nc.dram_tensor("w_inT", [E, E], MM_DT, kind="ExternalInput")
    w_outT = nc.dram_tensor("w_outT", [E, E], MM_DT, kind="ExternalInput")
    a_all = nc.dram_tensor(
        "a_all", [128, 3 * NCLS * 256], MM_DT, kind="ExternalInput"
    )
    wfl = nc.dram_tensor("wfl", [128, 2], DT, kind="ExternalInput")
    ones = nc.dram_tensor("ones", [1, 128], MM_DT, kind="ExternalInput")
    out = nc.dram_tensor("out", [L, E], DT, kind="ExternalOutput")

    with tile.TileContext(nc) as tc:
        with (
            tc.tile_pool(name="const", bufs=1) as cpool,
            tc.tile_pool(name="vbuf", bufs=1) as vpool,
            tc.tile_pool(name="outp", bufs=2) as outpool,
            tc.tile_pool(name="ps_big", bufs=2, space="PSUM") as ps_big,
            tc.tile_pool(name="ps_att", bufs=2, space="PSUM") as ps_att,
        ):
            # resident through phase 2
            w_outT_sb = cpool.tile([128, KT * E], MM_DT, tag="w_outT_sb")
            a_sb = cpool.tile([128, 3 * NCLS * 256], MM_DT, tag="a_sb")
            wfl_sb = cpool.tile([128, 2], DT, tag="wfl_sb")
            ones_sb = cpool.tile([1, 128], MM_DT, tag="ones_sb")
            v_sb = vpool.tile([128, NT * E], MM_DT, tag="v_sb")

            def ablk(t, ci):
                s = (t * NCLS + ci) * 256
                return a_sb[:, s : s + 256]

            # ---- phase 1: v[l-tile, m] = x @ W_in.T ----
            with (
                tc.tile_pool(name="w_in", bufs=1) as wpool,
                tc.tile_pool(name="xt", bufs=6) as xtpool,
            ):
                w_inT_sb = wpool.tile([128, KT * E], MM_DT, tag="w_inT_sb")

                def load_xt(i):
                    xt_t = xtpool.tile([128, KT * 128], MM_DT, tag="xt")
                    nc.sync.dma_start(
                        xt_t[:].rearrange("p (kt l) -> p kt l", kt=KT),
                        xT[:, i * 128 : (i + 1) * 128].rearrange(
                            "(kt p) l -> p kt l", p=128
                        ),
                    )
                    return xt_t

                # DMA issue order drives queue service order: the first
                # matmul needs only xt_0 + W_in[kt=0][:, :512].
                nc.sync.dma_start(
                    w_inT_sb[:, 0:512], w_inT[0:128, 0:512]
                )
                xt_first = load_xt(0)
                nc.sync.dma_start(
                    w_inT_sb[:, 512:E], w_inT[0:128, 512:E]
                )
                for kt in range(1, KT):
                    nc.sync.dma_start(
                        w_inT_sb[:, kt * E : (kt + 1) * E],
                        w_inT[kt * 128 : (kt + 1) * 128, :],
                    )

                for i in range(NT):
                    xt_t = xt_first if i == 0 else load_xt(i)
                    # interleave phase-2 table DMAs into the xt stream: early
                    # enough to be resident when phase 2 starts, late enough
                    # not to delay the phase-1 pipeline fill.
                    pv = ps_big.tile([128, E], DT, tag="pp")
                    # interior tiles skip v[:, 384:640]: heads 3/4 ('first'/
                    # 'last') only consume v rows {0:16, 2032:2048}, handled
                    # as a rank-1 update, so those columns are dead there.
                    if i in (0, NT - 1):
                        chunks = ((0, 512), (512, 512))
                    else:
                        chunks = ((0, 384), (640, 384))
                    for kt in range(KT):
                        lhsT = xt_t[:, kt * 128 : (kt + 1) * 128]
                        for m0, n in chunks:
                            nc.tensor.matmul(
                                pv[:, m0 : m0 + n],
                                lhsT,
                                w_inT_sb[:, kt * E + m0 : kt * E + m0 + n],
                                start=(kt == 0),
                                stop=(kt == KT - 1),
                            )
                    nc.vector.tensor_copy(v_sb[:, i * E : (i + 1) * E], pv[:])

            # phase-2 tables: issued after phase-1 DMAs so they don't delay
            # it; a_all first (attention consumes it before w_outT is needed)
            nc.sync.dma_start(a_sb[:], a_all[:])
            nc.sync.dma_start(wfl_sb[:], wfl[:])
            nc.sync.dma_start(ones_sb[:], ones[:])
            for kt in range(KT):
                nc.sync.dma_start(
                    w_outT_sb[:, kt * E : (kt + 1) * E],
                    w_outT[kt * 128 : (kt + 1) * 128, :],
                )

            if phases < 2:
                for i in range(NT):
                    ot = outpool.tile([128, E], DT, tag="out")
                    nc.scalar.copy(ot[:], v_sb[:, i * E : (i + 1) * E].bitcast(DT))
                    nc.sync.dma_start(out[i * 128 : (i + 1) * 128, :], ot[:])

            if phases >= 2:
                # ---- phase 2: per q-slab: attention, then output proj ----
                with tc.tile_pool(name="attp", bufs=2) as attpool:
                    for s in range(NSLAB):
                        att_sb = attpool.tile([128, NBH * 1024], MM_DT, tag="att")
                        for bi, h in enumerate(BANDED_HEADS):
                            t = HEAD_TYPE[h]
                            # collect this head's matmul pieces inside slab s
                            mms = []  # (col_in_slab, i, ci, bank)
                            for i in range(NT):
                                for start_rel, ci in _pieces(i):
                                    w0 = 128 * i + start_rel
                                    if not (1024 * s <= w0 < 1024 * (s + 1)):
                                        continue
                                    mms.append((w0 - 1024 * s, i, ci, w0 // 512))
                            last_of_bank = {}
                            for n_, mm in enumerate(mms):
                                last_of_bank[mm[3]] = n_
                            patt = ps_att.tile([128, 1024], DT, tag="patt")
                            started = set()
                            for n_, (col, i, ci, bank) in enumerate(mms):
                                first = bank not in started
                                started.add(bank)
                                nc.tensor.matmul(
                                    patt[:, col : col + 256],
                                    v_sb[:, i * E + h * 128 : i * E + (h + 1) * 128],
                                    ablk(t, ci),
                                    start=first,
                                    stop=(last_of_bank[bank] == n_),
                                )
                            nc.scalar.copy(
                                att_sb[:, bi * 1024 : (bi + 1) * 1024], patt[:]
                            )

                        if s == 0:
                            # ---- u vectors for 'first' (head 3) / 'last' (head 4) ----
                            pu = ps_big.tile([128, 2], DT, tag="pp")
                            nc.tensor.matmul(
                                pu[:, 0:1],
                                v_sb[:, 0 * E + 3 * 128 : 0 * E + 4 * 128].bitcast(DT),
                                wfl_sb[:, 0:1],
                                start=True,
                                stop=True,
                            )
                            nc.tensor.matmul(
                                pu[:, 1:2],
                                v_sb[:, 15 * E + 4 * 128 : 15 * E + 5 * 128].bitcast(DT),
                                wfl_sb[:, 1:2],
                                start=True,
                                stop=True,
                            )
                            u34_sb = cpool.tile([128, 2], MM_DT, tag="u34_sb")
                            nc.scalar.copy(u34_sb[:], pu[:])

                            # r34[e] = u3 @ W_outT[384:512, :] + u4 @ W_outT[512:640, :]
                            pr = ps_big.tile([1, E], DT, tag="pp")
                            for ec in range(2):
                                for hi, h in enumerate((3, 4)):
                                    nc.tensor.matmul(
                                        pr[:, ec * 512 : (ec + 1) * 512],
                                        u34_sb[:, hi : hi + 1],
                                        w_outT_sb[
                                            :, h * E + ec * 512 : h * E + ec * 512 + 512
                                        ],
                                        start=(hi == 0),
                                        stop=(hi == 1),
                                    )
                            r34_sb = cpool.tile([1, E], MM_DT, tag="r34_sb")
                            nc.scalar.copy(r34_sb[:], pr[:])

                            # broadcast r34 to all 128 partitions once (K=1 matmul),
                            # so the per-tile rank-1 update becomes a DVE add fused
                            # into the PSUM->SBUF out copy.
                            prb = ps_att.tile([128, 1024], DT, tag="patt")
                            for ec in range(2):
                                nc.tensor.matmul(
                                    prb[:, ec * 512 : (ec + 1) * 512],
                                    ones_sb[:],
                                    r34_sb[:, ec * 512 : (ec + 1) * 512],
                                    start=True,
                                    stop=True,
                                )
                            r34_full = cpool.tile([128, E], DT, tag="r34_full")
                            nc.scalar.copy(r34_full[:], prb[:])

                        for jj in range(8):  # q-tiles within slab
                            j = s * 8 + jj
                            po = ps_big.tile([128, E], DT, tag="pp")
                            for ec in range(2):
                                for bi, h in enumerate(BANDED_HEADS):
                                    nc.tensor.matmul(
                                        po[:, ec * 512 : (ec + 1) * 512],
                                        att_sb[
                                            :,
                                            bi * 1024
                                            + jj * 128 : bi * 1024
                                            + (jj + 1) * 128,
                                        ],
                                        w_outT_sb[
                                            :,
                                            h * E + ec * 512 : h * E + ec * 512 + 512,
                                        ],
                                        start=(bi == 0),
                                        stop=(bi == NBH - 1),
                                    )
                            out_t = outpool.tile([128, E], DT, tag="out")
                            nc.vector.tensor_add(out_t[:], po[:], r34_full[:])
                            nc.sync.dma_start(
                                out[j * 128 : (j + 1) * 128, :], out_t[:]
                            )

    nc.compile()
    return nc


class _Runner:
    """Builds the Bass program once and caches a jitted shard_map executable
    (one batch element per NeuronCore). Mirrors bass2jax.run_bass_via_pjrt
    but keeps the compiled callable + replicated weight arrays resident."""

    IN_ORDER = ["xT", "w_inT", "w_outT", "a_all", "wfl", "ones"]

    def __init__(self):
        import jax
        from jax.sharding import Mesh, PartitionSpec
        from jax.experimental.shard_map import shard_map

        self.jax = jax
        _b2j.install_neuronx_cc_hook()
        nc = _build_program()
        self.nc = nc
        self.a_all_np, self.wfl_np = _host_tables()

        partition_name = (
            nc.partition_id_tensor.name if nc.partition_id_tensor else None
        )
        in_names = []
        out_names = []
        out_avals = []
        for alloc in nc.m.functions[0].allocations:
            if not isinstance(alloc, mybir.MemoryLocationSet):
                continue
            name = alloc.memorylocations[0].name
            if alloc.kind == "ExternalInput":
                if name != partition_name:
                    in_names.append(name)
            elif alloc.kind == "ExternalOutput":
                out_names.append(name)
                out_avals.append(
                    jax.core.ShapedArray(
                        tuple(alloc.tensor_shape), mybir.dt.np(alloc.dtype)
                    )
                )
        assert sorted(in_names) == sorted(self.IN_ORDER), in_names
        self.in_names = in_names
        self.out_names = out_names
        self.out_avals = out_avals
        n_params = len(in_names)
        n_outs = len(out_names)
        all_names = tuple(in_names) + tuple(out_names)
        if partition_name is not None:
            all_names = all_names + (partition_name,)

        def _body(*args):
            operands = list(args)
            if partition_name is not None:
                operands.append(_b2j.partition_id_tensor())
            outs = _b2j._bass_exec_p.bind(
                *operands,
                out_avals=tuple(out_avals),
                in_names=all_names,
                out_names=tuple(out_names),
                lowering_input_output_aliases=(),
                sim_require_finite=True,
                sim_require_nnan=True,
                nc=nc,
            )
            return tuple(outs)

        devices = jax.devices()[:B]
        assert len(devices) == B
        self.mesh = Mesh(np.asarray(devices), ("core",))
        in_specs = (PartitionSpec("core"),) * (n_params + n_outs)
        out_specs = (PartitionSpec("core"),) * n_outs
        self.sharded = jax.jit(
            shard_map(
                _body,
                mesh=self.mesh,
                in_specs=in_specs,
                out_specs=out_specs,
                check_rep=False,
            ),
            donate_argnums=tuple(range(n_params, n_params + n_outs)),
            keep_unused=True,
        )

    def _concat_static(self, w_inT, w_outT):
        jax = self.jax
        statics = {
            "w_inT": w_inT,
            "w_outT": w_outT,
            "a_all": self.a_all_np,
            "wfl": self.wfl_np,
            "ones": np.ones((1, 128), dtype=np.float32),
        }
        out = {}
        for name, arr in statics.items():
            big = np.concatenate([arr] * B, axis=0)
            out[name] = jax.device_put(big)
        return out

    def run_device(self, dev_args):
        jnp = self.jax.numpy
        zeros = [
            jnp.zeros((B * av.shape[0], *av.shape[1:]), av.dtype)
            for av in self.out_avals
        ]
        return self.sharded(*dev_args, *zeros)

    def prepare_inputs(self, x, W_in, W_out):
        xT_np = np.ascontiguousarray(x.transpose(0, 2, 1)).reshape(B * E, L)
        w_inT_np = np.ascontiguousarray(W_in.T)
        w_outT_np = np.ascontiguousarray(W_out.T)
        dev = self._concat_static(w_inT_np, w_outT_np)
        dev["xT"] = self.jax.device_put(xT_np)
        return [dev[name] for name in self.in_names]

    def __call__(self, x, W_in, W_out):
        args = self.prepare_inputs(x, W_in, W_out)
        outs = self.run_device(args)
        out = np.asarray(outs[self.out_names.index("out")])
        return out.reshape(B, L, E)


_CACHE = {}


def _get_runner() -> _Runner:
    if "runner" not in _CACHE:
        _CACHE["runner"] = _Runner()
    return _CACHE["runner"]


def kernel(x, W_in, W_out):
    x = np.ascontiguousarray(np.asarray(x, dtype=np.float32))
    W_in = np.ascontiguousarray(np.asarray(W_in, dtype=np.float32))
    W_out = np.ascontiguousarray(np.asarray(W_out, dtype=np.float32))
    assert x.shape == (B, L, E)
    return _get_runner()(x, W_in, W_out)


if __name__ == "__main__":
    rng = np.random.default_rng(0)
    x = rng.standard_normal((B, L, E), dtype=np.float32)
    W_in = rng.standard_normal((E, E), dtype=np.float32) * 0.05
    W_out = rng.standard_normal((E, E), dtype=np.float32) * 0.05
    y = kernel(x, W_in, W_out)
    print("out", y.shape, y.dtype, np.abs(y).mean())



# revision 5
# speedup vs baseline: 1.3367x; 1.3367x over previous
"""Trainium2 Bass kernel for nn_NewAttention (analytic Gaussian sparse attention).

Math (per batch element b):
    v        = x[b] @ W_in.T                      # [L, E]
    per head h (P=128 cols of v):
        A_h  = softmax(-(j - c_h(i))^2 / 2)       # [L, L], analytic, banded
        att_h = A_h @ v_h                         # [L, P]
    out[b]   = concat_h(att_h) @ W_out.T          # [L, E]

Sharding: data-parallel over batch, one batch element per NeuronCore (8 cores).

Device strategy (per core):
  - mm1 (v = x @ W_in.T) runs as 3 fp8-e4m3 DoubleRow passes (main +
    x-residual + W-residual; the dropped cross term is ~1e-3 relative).
    Host pre-quantizes x*32 and W_in.T*2048 plus their e4m3 residuals and
    lays both out in the DoubleRow [128, 2, .] pairing. PSUM accumulates all
    3 passes; a DVE copy scales by 1/65536 into bf16 v_sb (natural units).
  - attention: att^T_h = v_h.T @ (A^T * 32) as banded bf16 matmuls with
    144-wide analytic weight blocks (one interior block per head type plus
    boundary-renormalized first/last blocks; band halo +-7, truncation
    ~1e-14). PSUM accumulates overlapping windows via has_written bits.
    patt (= att*32) is copied twice: Act -> fp8 att_hi, DVE tensor_sub ->
    fp8 att_lo residual.
  - mm2 produces out^T (partition = embed dim): 3 fp8 DoubleRow passes
    (hi@W8o + lo@W8o + hi@dW8o) over 3 banded-head pairs. The PSUM->SBUF
    copy is a fused Activation Identity(po*1/65536 + r34T bias), where the
    per-partition bias vector r34T carries the rank-1 'first'/'last' head
    contribution (they attend to a fixed key location for every query).
  - output DMA'd as out^T [E, L]; host transposes back.
"""

import sys
import numpy as np

for _p in ("/opt/trn_rl_repo",):
    if _p not in sys.path:
        sys.path.insert(0, _p)

import concourse.bass as bass
import concourse.bacc as bacc
import concourse.mybir as mybir
from concourse import tile
from concourse import bass2jax as _b2j
import ml_dtypes

# ---------------- problem constants (hardcoded per contract) ----------------
B = 8
L = 2048
E = 1024
H = 8
P = 128
SIGMA = 1.0
DISP = 1
NT = L // 128           # 16 l-tiles
KT2 = E // 256          # 4 DoubleRow contraction groups
DT = mybir.dt.float32
BF = mybir.dt.bfloat16
F8 = mybir.dt.float8e4
DR = mybir.MatmulPerfMode.DoubleRow
NPF8 = ml_dtypes.float8_e4m3
NPBF = ml_dtypes.bfloat16

SX = 32.0        # x fp8 scale
SWI = 2048.0     # W_in fp8 scale
SA = 32.0        # att fp8 scale (folded into A table)
SWO = 2048.0     # W_out fp8 scale
INV1 = 1.0 / (SX * SWI)
INV2 = 1.0 / (SA * SWO)

BANDED_HEADS = [0, 1, 2, 5, 6, 7]   # center,left,right,center,left,right
NBH = len(BANDED_HEADS)
BTYPE = [0, 1, 2, 0, 1, 2]          # per banded idx: 0=center,1=left,2=right
TYPE_DISP = [0, -DISP, +DISP]
VW = NBH * 128                      # 768 banded v cols per tile
# A-table layout per type: [interior 144 | first 136 | last 136]
AT_INT, AT_FIRST, AT_LAST, AT_STRIDE = 0, 144, 280, 416


def _att_pieces(i):
    """Attention pieces for v-tile i: (q0, q1, block_col0, block_base_off).

    Window of q positions tile i contributes to (halo +-7 around the tile,
    clipped at sequence bounds), split at PSUM 512-col bank boundaries.
    """
    if i == 0:
        w0, wid, base = 0, 136, AT_FIRST
    elif i == NT - 1:
        w0, wid, base = 128 * i - 8, 136, AT_LAST
    else:
        w0, wid, base = 128 * i - 8, 144, AT_INT
    out = []
    q = w0
    while q < w0 + wid:
        qe = min(w0 + wid, (q // 512 + 1) * 512)
        out.append((q, qe, q - w0, base))
        q = qe
    return out


def _softmax_rows(logits):
    m = logits.max(axis=-1, keepdims=True)
    e = np.exp(logits - m)
    return e / e.sum(axis=-1, keepdims=True)


def _host_tables():
    """Analytic attention weight blocks (exact, float64 -> bf16, scaled by SA)
    and the first/last-head key-weight vectors."""
    j = np.arange(L, dtype=np.float64)
    i = np.arange(L, dtype=np.float64)

    a_tab = np.zeros((128, 3 * AT_STRIDE), dtype=np.float64)
    for t, disp in enumerate(TYPE_DISP):
        c = i + disp
        logits = -((j[None, :] - c[:, None]) ** 2) / (2.0 * SIGMA**2)
        A = _softmax_rows(logits)  # [Lq, Lk]
        base = t * AT_STRIDE
        # interior block from representative tile 4: B[p, c] = A[504+c, 512+p]
        a_tab[:, base + AT_INT:base + AT_INT + 144] = A[504:648, 512:640].T
        a_tab[:, base + AT_FIRST:base + AT_FIRST + 136] = A[0:136, 0:128].T
        a_tab[:, base + AT_LAST:base + AT_LAST + 136] = A[1912:2048, 1920:2048].T
    a_tab *= SA

    Af = _softmax_rows(-((j[None, :]) ** 2) / (2 * SIGMA**2))
    Al = _softmax_rows(-((j[None, :] - (L - 1.0)) ** 2) / (2 * SIGMA**2))
    wfl = np.zeros((128, 2), dtype=np.float64)
    wfl[:, 0] = Af[0, 0:128]         # 'first': support at k < 128 (v tile 0)
    wfl[:, 1] = Al[0, L - 128:L]     # 'last': support at k >= L-128 (tile 15)

    return a_tab.astype(NPBF), wfl.astype(NPBF)


def _build_program():
    nc = bacc.Bacc("TRN2", target_bir_lowering=False, debug=False, num_devices=B)

    x8 = nc.dram_tensor("x8", [128, NT * 1024], F8, kind="ExternalInput")
    dx8 = nc.dram_tensor("dx8", [128, NT * 1024], F8, kind="ExternalInput")
    w8i = nc.dram_tensor("w8i", [128, KT2 * 2048], F8, kind="ExternalInput")
    dw8i = nc.dram_tensor("dw8i", [128, KT2 * 2048], F8, kind="ExternalInput")
    w8o = nc.dram_tensor("w8o", [128, NBH * 1024], F8, kind="ExternalInput")
    dw8o = nc.dram_tensor("dw8o", [128, NBH * 1024], F8, kind="ExternalInput")
    a_tab = nc.dram_tensor("a_tab", [128, 3 * AT_STRIDE], BF, kind="ExternalInput")
    w34 = nc.dram_tensor("w34", [128, 2 * E], BF, kind="ExternalInput")
    wfl = nc.dram_tensor("wfl", [128, 2], BF, kind="ExternalInput")
    out = nc.dram_tensor("out", [E, L], DT, kind="ExternalOutput")

    with tile.TileContext(nc) as tc:
        with (
            tc.tile_pool(name="const", bufs=1) as cpool,
            tc.tile_pool(name="vbuf", bufs=1) as vpool,
            tc.tile_pool(name="attb", bufs=2) as attpool,
            tc.tile_pool(name="outp", bufs=4) as outpool,
            tc.tile_pool(name="ps_att", bufs=2, space="PSUM") as ps_att,
        ):
            x8_sb = cpool.tile([128, NT * 1024], F8, tag="x8_sb")
            dx8_sb = cpool.tile([128, NT * 1024], F8, tag="dx8_sb")
            w8i_sb = cpool.tile([128, KT2 * 2048], F8, tag="w8i_sb")
            dw8i_sb = cpool.tile([128, KT2 * 2048], F8, tag="dw8i_sb")
            w8o_sb = cpool.tile([128, NBH * 1024], F8, tag="w8o_sb")
            dw8o_sb = cpool.tile([128, NBH * 1024], F8, tag="dw8o_sb")
            a_sb = cpool.tile([128, 3 * AT_STRIDE], BF, tag="a_sb")
            w34_sb = cpool.tile([128, 2 * E], BF, tag="w34_sb")
            wfl_sb = cpool.tile([128, 2], BF, tag="wfl_sb")
            v_sb = vpool.tile([128, NT * VW], BF, tag="v_sb")
            vf_sb = cpool.tile([128, 128], BF, tag="vf_sb")
            vl_sb = cpool.tile([128, 128], BF, tag="vl_sb")
            u34_sb = cpool.tile([128, 2], BF, tag="u34_sb")
            r34_sb = cpool.tile([128, 8], DT, tag="r34_sb")

            # ---- DMA issue order drives queue service order ----
            nc.sync.dma_start(w8i_sb[:, 0:2048], w8i[:, 0:2048])
            nc.sync.dma_start(x8_sb[:, 0:1024], x8[:, 0:1024])
            nc.sync.dma_start(dx8_sb[:, 0:1024], dx8[:, 0:1024])
            nc.sync.dma_start(w8i_sb[:, 2048:], w8i[:, 2048:])
            nc.sync.dma_start(dw8i_sb[:], dw8i[:])
            nc.sync.dma_start(x8_sb[:, 1024:2048], x8[:, 1024:2048])
            nc.sync.dma_start(dx8_sb[:, 1024:2048], dx8[:, 1024:2048])
            nc.sync.dma_start(x8_sb[:, 2048:9216], x8[:, 2048:9216])
            nc.sync.dma_start(dx8_sb[:, 2048:9216], dx8[:, 2048:9216])
            nc.sync.dma_start(a_sb[:], a_tab[:])
            nc.sync.dma_start(wfl_sb[:], wfl[:])
            nc.sync.dma_start(x8_sb[:, 9216:], x8[:, 9216:])
            nc.sync.dma_start(dx8_sb[:, 9216:], dx8[:, 9216:])
            nc.sync.dma_start(w34_sb[:], w34[:])
            nc.sync.dma_start(w8o_sb[:], w8o[:])
            nc.sync.dma_start(dw8o_sb[:], dw8o[:])

            def mm1_tile(ps_v, i):
                """3-pass fp8 DoubleRow accumulation of v-tile i into PSUM,
                then DVE copy (scale 1/65536) into bf16 v_sb."""
                if i == 0:
                    chunks = ((0, 512), (640, 384))
                elif i == NT - 1:
                    chunks = ((0, 384), (512, 512))
                else:
                    chunks = ((0, 384), (640, 384))
                pv = ps_v.tile([128, 1024], DT, tag="pv")
                passes = ((x8_sb, w8i_sb), (dx8_sb, w8i_sb), (x8_sb, dw8i_sb))
                for p, (xa, wa) in enumerate(passes):
                    for kt2 in range(KT2):
                        lhsT = xa[:, i * 1024 + kt2 * 256:i * 1024 + (kt2 + 1) * 256]
                        lhsT = lhsT.rearrange("p (s l) -> p s l", s=2)
                        wview = wa[:, kt2 * 2048:(kt2 + 1) * 2048]
                        wview = wview.rearrange("p (s m) -> p s m", s=2)
                        for m0, n in chunks:
                            nc.tensor.matmul(
                                pv[:, m0:m0 + n],
                                lhsT,
                                wview[:, :, m0:m0 + n],
                                start=(p == 0 and kt2 == 0),
                                stop=(p == 2 and kt2 == KT2 - 1),
                                perf_mode=DR,
                            )
                # banded head cols -> v_sb (natural units, bf16)
                nc.vector.tensor_scalar_mul(
                    v_sb[:, i * VW:i * VW + 384], pv[:, 0:384], INV1
                )
                nc.vector.tensor_scalar_mul(
                    v_sb[:, i * VW + 384:(i + 1) * VW], pv[:, 640:1024], INV1
                )
                if i == 0:
                    nc.vector.tensor_scalar_mul(vf_sb[:], pv[:, 384:512], INV1)
                if i == NT - 1:
                    nc.vector.tensor_scalar_mul(vl_sb[:], pv[:, 512:640], INV1)

            def attn_slab(s, att_hi, att_lo):
                """Banded attention for q-slab s: per head, accumulate banded
                bf16 matmul pieces into patt (= att*SA), then split to fp8
                hi (Act copy) + lo residual (DVE tensor_sub)."""
                for bi in range(NBH):
                    t = BTYPE[bi]
                    mms = []
                    for i in range(NT):
                        for q0, q1, c0, base in _att_pieces(i):
                            if not (1024 * s <= q0 < 1024 * (s + 1)):
                                continue
                            mms.append((q0, q1, c0, base, i, (q0 - 1024 * s) // 512))
                    last_of_bank = {}
                    for n_, mm in enumerate(mms):
                        last_of_bank[mm[5]] = n_
                    patt = ps_att.tile([128, 1024], DT, tag="patt")
                    started = set()
                    for n_, (q0, q1, c0, base, i, bank) in enumerate(mms):
                        first = bank not in started
                        started.add(bank)
                        col = t * AT_STRIDE + base + c0
                        nc.tensor.matmul(
                            patt[:, q0 - 1024 * s:q1 - 1024 * s],
                            v_sb[:, i * VW + bi * 128:i * VW + (bi + 1) * 128],
                            a_sb[:, col:col + (q1 - q0)],
                            start=first,
                            stop=(last_of_bank[bank] == n_),
                        )
                    hi = att_hi[:, bi * 1024:(bi + 1) * 1024]
                    nc.scalar.copy(hi, patt[:])
                    nc.vector.tensor_sub(
                        att_lo[:, bi * 1024:(bi + 1) * 1024], patt[:], hi
                    )

            att_hi = [None, None]
            att_lo = [None, None]

            with tc.tile_pool(name="ps_v", bufs=2, space="PSUM") as ps_v:
                for i in range(9):
                    mm1_tile(ps_v, i)

                att_hi[0] = attpool.tile([128, NBH * 1024], F8, tag="hi", name="hi0")
                att_lo[0] = attpool.tile([128, NBH * 1024], F8, tag="lo", name="lo0")
                attn_slab(0, att_hi[0], att_lo[0])

                for i in range(9, NT):
                    mm1_tile(ps_v, i)

                # ---- rank-1 'first'/'last' head correction, as out^T bias ----
                # u3 = wf @ v3 (tile 0), u4 = wl @ v4 (tile 15)   [128 m, 1]
                pu = ps_v.tile([128, 1024], DT, tag="pv")
                nc.tensor.matmul(pu[:, 0:1], vf_sb[:], wfl_sb[:, 0:1],
                                 start=True, stop=True)
                nc.tensor.matmul(pu[:, 1:2], vl_sb[:], wfl_sb[:, 1:2],
                                 start=True, stop=True)
                nc.scalar.copy(u34_sb[:], pu[:, 0:2])
                # r34T[e] = sum_m W_outT[384+m, e] u3[m] + W_outT[512+m, e] u4[m]
                pr = ps_v.tile([128, 1024], DT, tag="pv")
                for t in range(8):
                    for hh in range(2):
                        nc.tensor.matmul(
                            pr[:, t:t + 1],
                            w34_sb[:, hh * E + t * 128:hh * E + (t + 1) * 128],
                            u34_sb[:, hh:hh + 1],
                            start=(hh == 0),
                            stop=(hh == 1),
                        )
                nc.scalar.copy(r34_sb[:], pr[:, 0:8])

            att_hi[1] = attpool.tile([128, NBH * 1024], F8, tag="hi", name="hi1")
            att_lo[1] = attpool.tile([128, NBH * 1024], F8, tag="lo", name="lo1")
            attn_slab(1, att_hi[1], att_lo[1])

            # ---- mm2: out^T[e, q] via 3 fp8 DR passes over 3 head pairs ----
            with tc.tile_pool(name="ps_o", bufs=4, space="PSUM") as ps_o:
                for c in range(4):          # q-chunks of 512
                    s = c // 2
                    hi_v = att_hi[s][:].rearrange("p (bi q) -> p bi q", bi=NBH)
                    lo_v = att_lo[s][:].rearrange("p (bi q) -> p bi q", bi=NBH)
                    q0 = (c % 2) * 512
                    for t in range(8):      # e-tiles
                        po = ps_o.tile([128, 512], DT, tag="po")
                        for p, av in enumerate((hi_v, lo_v, hi_v)):
                            wv = (w8o_sb if p < 2 else dw8o_sb)[:].rearrange(
                                "p (bi e) -> p bi e", bi=NBH
                            )
                            for hp in range(3):
                                nc.tensor.matmul(
                                    po[:],
                                    wv[:, 2 * hp:2 * hp + 2,
                                       t * 128:(t + 1) * 128],
                                    av[:, 2 * hp:2 * hp + 2, q0:q0 + 512],
                                    start=(p == 0 and hp == 0),
                                    stop=(p == 2 and hp == 2),
                                    perf_mode=DR,
                                )
                        ot = outpool.tile([128, 512], DT, tag="out")
                        nc.scalar.activation(
                            ot[:], po[:],
                            mybir.ActivationFunctionType.Identity,
                            bias=r34_sb[:, t:t + 1], scale=INV2,
                        )
                        nc.sync.dma_start(
                            out[t * 128:(t + 1) * 128,
                                c * 512:(c + 1) * 512],
                            ot[:],
                        )

    nc.compile()
    return nc


class _Runner:
    """Builds the Bass program once and caches a jitted shard_map executable
    (one batch element per NeuronCore)."""

    IN_ORDER = ["x8", "dx8", "w8i", "dw8i", "w8o", "dw8o", "a_tab", "w34", "wfl"]

    def __init__(self):
        import jax
        from jax.sharding import Mesh, PartitionSpec
        from jax.experimental.shard_map import shard_map

        self.jax = jax
        _b2j.install_neuronx_cc_hook()
        nc = _build_program()
        self.nc = nc
        self.a_tab_np, self.wfl_np = _host_tables()

        partition_name = (
            nc.partition_id_tensor.name if nc.partition_id_tensor else None
        )
        in_names = []
        out_names = []
        out_avals = []
        for alloc in nc.m.functions[0].allocations:
            if not isinstance(alloc, mybir.MemoryLocationSet):
                continue
            name = alloc.memorylocations[0].name
            if alloc.kind == "ExternalInput":
                if name != partition_name:
                    in_names.append(name)
            elif alloc.kind == "ExternalOutput":
                out_names.append(name)
                out_avals.append(
                    jax.core.ShapedArray(
                        tuple(alloc.tensor_shape), mybir.dt.np(alloc.dtype)
                    )
                )
        assert sorted(in_names) == sorted(self.IN_ORDER), in_names
        self.in_names = in_names
        self.out_names = out_names
        self.out_avals = out_avals
        n_params = len(in_names)
        n_outs = len(out_names)
        all_names = tuple(in_names) + tuple(out_names)
        if partition_name is not None:
            all_names = all_names + (partition_name,)

        def _body(*args):
            operands = list(args)
            if partition_name is not None:
                operands.append(_b2j.partition_id_tensor())
            outs = _b2j._bass_exec_p.bind(
                *operands,
                out_avals=tuple(out_avals),
                in_names=all_names,
                out_names=tuple(out_names),
                lowering_input_output_aliases=(),
                sim_require_finite=True,
                sim_require_nnan=True,
                nc=nc,
            )
            return tuple(outs)

        devices = jax.devices()[:B]
        assert len(devices) == B
        self.mesh = Mesh(np.asarray(devices), ("core",))
        in_specs = (PartitionSpec("core"),) * (n_params + n_outs)
        out_specs = (PartitionSpec("core"),) * n_outs
        self.sharded = jax.jit(
            shard_map(
                _body,
                mesh=self.mesh,
                in_specs=in_specs,
                out_specs=out_specs,
                check_rep=False,
            ),
            donate_argnums=tuple(range(n_params, n_params + n_outs)),
            keep_unused=True,
        )

    def _concat_static(self, statics):
        jax = self.jax
        out = {}
        for name, arr in statics.items():
            big = np.concatenate([arr] * B, axis=0)
            out[name] = jax.device_put(big)
        return out

    def run_device(self, dev_args):
        jnp = self.jax.numpy
        zeros = [
            jnp.zeros((B * av.shape[0], *av.shape[1:]), av.dtype)
            for av in self.out_avals
        ]
        return self.sharded(*dev_args, *zeros)

    def prepare_inputs(self, x, W_in, W_out):
        # ---- x: per batch, 2-level e4m3 at scale SX, DoubleRow layout ----
        xs = x.reshape(B, L, E) * np.float32(SX)
        x8 = xs.astype(NPF8)
        dx8 = (xs - x8.astype(np.float32)).astype(NPF8)

        def dr_x(a8):  # [B, L, E] fp8 -> [B*128, NT*1024]
            t = a8.reshape(B, NT, 128, KT2, 2, 128)   # b, i, l, kt2, s, p
            t = t.transpose(0, 5, 1, 3, 4, 2)         # b, p, i, kt2, s, l
            return np.ascontiguousarray(t).reshape(B * 128, NT * 1024)

        # ---- W_in.T: 2-level e4m3 at scale SWI, DoubleRow layout ----
        wiT = W_in.T * np.float32(SWI)
        w8 = wiT.astype(NPF8)
        dw8 = (wiT - w8.astype(np.float32)).astype(NPF8)

        def dr_wi(a8):  # [E, E] fp8 -> [128, KT2*2048]
            t = a8.reshape(KT2, 2, 128, E)            # kt2, s, p, m
            t = t.transpose(2, 0, 1, 3)               # p, kt2, s, m
            return np.ascontiguousarray(t).reshape(128, KT2 * 2048)

        # ---- W_out.T banded rows: 2-level e4m3 at scale SWO, pair layout ----
        woT = W_out.T * np.float32(SWO)
        wo8 = woT.astype(NPF8)
        dwo8 = (woT - wo8.astype(np.float32)).astype(NPF8)

        def dr_wo(a8):  # [E, E] fp8 -> [128, NBH*1024]
            t = np.stack([a8[h * 128:(h + 1) * 128, :] for h in BANDED_HEADS])
            t = t.transpose(1, 0, 2)                  # p, bi, e
            return np.ascontiguousarray(t).reshape(128, NBH * E)

        # ---- W_out.T rows for heads 3/4 (bf16, natural units) ----
        w34 = (W_out.T[384:640, :]).reshape(2, 128, E).transpose(1, 0, 2)
        w34 = np.ascontiguousarray(w34).reshape(128, 2 * E).astype(NPBF)

        statics = {
            "w8i": dr_wi(w8),
            "dw8i": dr_wi(dw8),
            "w8o": dr_wo(wo8),
            "dw8o": dr_wo(dwo8),
            "a_tab": self.a_tab_np,
            "w34": w34,
            "wfl": self.wfl_np,
        }
        dev = self._concat_static(statics)
        dev["x8"] = self.jax.device_put(dr_x(x8))
        dev["dx8"] = self.jax.device_put(dr_x(dx8))
        return [dev[name] for name in self.in_names]

    def __call__(self, x, W_in, W_out):
        args = self.prepare_inputs(x, W_in, W_out)
        outs = self.run_device(args)
        outT = np.asarray(outs[self.out_names.index("out")])  # [B*E, L]
        return np.ascontiguousarray(outT.reshape(B, E, L).transpose(0, 2, 1))


_CACHE = {}


def _get_runner() -> _Runner:
    if "runner" not in _CACHE:
        _CACHE["runner"] = _Runner()
    return _CACHE["runner"]


def kernel(x, W_in, W_out):
    x = np.ascontiguousarray(np.asarray(x, dtype=np.float32))
    W_in = np.ascontiguousarray(np.asarray(W_in, dtype=np.float32))
    W_out = np.ascontiguousarray(np.asarray(W_out, dtype=np.float32))
    assert x.shape == (B, L, E)
    return _get_runner()(x, W_in, W_out)


if __name__ == "__main__":
    rng = np.random.default_rng(0)
    x = rng.standard_normal((B, L, E), dtype=np.float32)
    W_in = rng.standard_normal((E, E), dtype=np.float32) * 0.05
    W_out = rng.standard_normal((E, E), dtype=np.float32) * 0.05
    y = kernel(x, W_in, W_out)
    print("out", y.shape, y.dtype, np.abs(y).mean())


# revision 49
# speedup vs baseline: 1.4544x; 1.0880x over previous
"""Trainium2 Bass kernel for nn_NewAttention (analytic Gaussian sparse attention).

Math (per batch element b):
    v        = x[b] @ W_in.T                      # [L, E]
    per head h (P=128 cols of v):
        A_h  = softmax(-(j - c_h(i))^2 / 2)       # [L, L], analytic, banded
        att_h = A_h @ v_h                         # [L, P]
    out[b]   = concat_h(att_h) @ W_out.T          # [L, E]

Sharding: data-parallel over batch, one batch element per NeuronCore (8 cores).

Device strategy (per core):
  - mm1 (v = x @ W_in.T) runs as 3 fp8-e4m3 DoubleRow passes (main +
    x-residual + W-residual; the dropped cross term is ~1e-3 relative).
    Host pre-quantizes x*32 and W_in.T*2048 plus their e4m3 residuals and
    lays both out in the DoubleRow [128, 2, .] pairing. PSUM accumulates all
    3 passes; a DVE copy scales by 1/65536 into bf16 v_sb (natural units).
  - attention: att^T_h = v_h.T @ (A^T * 32) as banded bf16 matmuls with
    144-wide analytic weight blocks (one interior block per head type plus
    boundary-renormalized first/last blocks; band halo +-7, truncation
    ~1e-14). PSUM accumulates overlapping windows via has_written bits.
    patt (= att*32) is copied twice: Act -> fp8 att_hi, DVE tensor_sub ->
    fp8 att_lo residual.
  - mm2 produces out^T (partition = embed dim): 3 fp8 DoubleRow passes
    (hi@W8o + lo@W8o + hi@dW8o) over 3 banded-head pairs. The PSUM->SBUF
    copy is a fused Activation Identity(po*1/65536 + r34T bias), where the
    per-partition bias vector r34T carries the rank-1 'first'/'last' head
    contribution (they attend to a fixed key location for every query).
  - output DMA'd as out^T [E, L]; host transposes back.
"""

import sys
import numpy as np

for _p in ("/opt/trn_rl_repo",):
    if _p not in sys.path:
        sys.path.insert(0, _p)

import concourse.bass as bass
import concourse.bacc as bacc
import concourse.mybir as mybir
from concourse import tile
from concourse import bass2jax as _b2j
import ml_dtypes

# ---------------- problem constants (hardcoded per contract) ----------------
B = 8
L = 2048
E = 1024
H = 8
P = 128
SIGMA = 1.0
DISP = 1
NT = L // 128           # 16 l-tiles
KT2 = E // 256          # 4 DoubleRow contraction groups
DT = mybir.dt.float32
BF = mybir.dt.bfloat16
F8 = mybir.dt.float8e4
DR = mybir.MatmulPerfMode.DoubleRow
NPF8 = ml_dtypes.float8_e4m3
NPBF = ml_dtypes.bfloat16

SX = 32.0        # x fp8 scale
SWI = 2048.0     # W_in fp8 scale
SA = 32.0        # att fp8 scale (folded into A table)
SWO = 2048.0     # W_out fp8 scale
INV1 = 1.0 / (SX * SWI)
INV2 = 1.0 / (SA * SWO)

WARM = 12        # PE warmup matmuls during initial DMA fill

BANDED_HEADS = [0, 1, 2, 5, 6, 7]   # center,left,right,center,left,right
NBH = len(BANDED_HEADS)
BTYPE = [0, 1, 2, 0, 1, 2]          # per banded idx: 0=center,1=left,2=right
TYPE_DISP = [0, -DISP, +DISP]
VW = NBH * 128                      # 768 banded v cols per tile
# A-table layout per type: [interior 144 | first 136 | last 136]
AT_INT, AT_FIRST, AT_LAST, AT_STRIDE = 0, 144, 280, 416


def _att_pieces(i):
    """Attention pieces for v-tile i: (q0, q1, block_col0, block_base_off).

    Window of q positions tile i contributes to (halo +-7 around the tile,
    clipped at sequence bounds), split at PSUM 512-col bank boundaries.
    """
    if i == 0:
        w0, wid, base = 0, 136, AT_FIRST
    elif i == NT - 1:
        w0, wid, base = 128 * i - 8, 136, AT_LAST
    else:
        w0, wid, base = 128 * i - 8, 144, AT_INT
    out = []
    q = w0
    while q < w0 + wid:
        qe = min(w0 + wid, (q // 512 + 1) * 512)
        out.append((q, qe, q - w0, base))
        q = qe
    return out


def _softmax_rows(logits):
    m = logits.max(axis=-1, keepdims=True)
    e = np.exp(logits - m)
    return e / e.sum(axis=-1, keepdims=True)


def _host_tables():
    """Analytic attention weight blocks (exact, float64 -> bf16, scaled by SA)
    and the first/last-head key-weight vectors."""
    j = np.arange(L, dtype=np.float64)
    i = np.arange(L, dtype=np.float64)

    a_tab = np.zeros((128, 3 * AT_STRIDE), dtype=np.float64)
    for t, disp in enumerate(TYPE_DISP):
        c = i + disp
        logits = -((j[None, :] - c[:, None]) ** 2) / (2.0 * SIGMA**2)
        A = _softmax_rows(logits)  # [Lq, Lk]
        base = t * AT_STRIDE
        # interior block from representative tile 4: B[p, c] = A[504+c, 512+p]
        a_tab[:, base + AT_INT:base + AT_INT + 144] = A[504:648, 512:640].T
        a_tab[:, base + AT_FIRST:base + AT_FIRST + 136] = A[0:136, 0:128].T
        a_tab[:, base + AT_LAST:base + AT_LAST + 136] = A[1912:2048, 1920:2048].T
    a_tab *= SA

    Af = _softmax_rows(-((j[None, :]) ** 2) / (2 * SIGMA**2))
    Al = _softmax_rows(-((j[None, :] - (L - 1.0)) ** 2) / (2 * SIGMA**2))
    wfl = np.zeros((128, 2), dtype=np.float64)
    wfl[:, 0] = Af[0, 0:128]         # 'first': support at k < 128 (v tile 0)
    wfl[:, 1] = Al[0, L - 128:L]     # 'last': support at k >= L-128 (tile 15)

    # v_sb is held in scaled units (v * SX*SWI); fold the descale into the
    # attention table so the v copy is a plain TensorCopy.
    a_tab *= INV1
    return a_tab.astype(NPBF), wfl.astype(NPBF)


def _build_program():
    nc = bacc.Bacc("TRN2", target_bir_lowering=False, debug=False, num_devices=B)

    x8 = nc.dram_tensor("x8", [128, NT * 1024], F8, kind="ExternalInput")
    dx8 = nc.dram_tensor("dx8", [128, NT * 1024], F8, kind="ExternalInput")
    w8i = nc.dram_tensor("w8i", [128, KT2 * 2048], F8, kind="ExternalInput")
    dw8i = nc.dram_tensor("dw8i", [128, KT2 * 2048], F8, kind="ExternalInput")
    w8o = nc.dram_tensor("w8o", [128, NBH * 1024], F8, kind="ExternalInput")
    dw8o = nc.dram_tensor("dw8o", [128, NBH * 1024], F8, kind="ExternalInput")
    a_tab = nc.dram_tensor("a_tab", [128, 3 * AT_STRIDE], BF, kind="ExternalInput")
    w34 = nc.dram_tensor("w34", [128, 2 * E], BF, kind="ExternalInput")
    wfl = nc.dram_tensor("wfl", [128, 2], BF, kind="ExternalInput")
    out = nc.dram_tensor("out", [E, L], DT, kind="ExternalOutput")

    with tile.TileContext(nc) as tc:
        with (
            tc.tile_pool(name="const", bufs=1) as cpool,
            tc.tile_pool(name="vbuf", bufs=1) as vpool,
            tc.tile_pool(name="attb", bufs=2) as attpool,
            tc.tile_pool(name="outp", bufs=6) as outpool,
            tc.tile_pool(name="ps_att", bufs=2, space="PSUM") as ps_att,
        ):
            x8_sb = cpool.tile([128, NT * 1024], F8, tag="x8_sb")
            dx8_sb = cpool.tile([128, NT * 1024], F8, tag="dx8_sb")
            w8i_sb = cpool.tile([128, KT2 * 2048], F8, tag="w8i_sb")
            dw8i_sb = cpool.tile([128, KT2 * 2048], F8, tag="dw8i_sb")
            w8o_sb = cpool.tile([128, NBH * 1024], F8, tag="w8o_sb")
            dw8o_sb = cpool.tile([128, NBH * 1024], F8, tag="dw8o_sb")
            a_sb = cpool.tile([128, 3 * AT_STRIDE], BF, tag="a_sb")
            w34_sb = cpool.tile([128, 2 * E], BF, tag="w34_sb")
            wfl_sb = cpool.tile([128, 2], BF, tag="wfl_sb")
            v_sb = vpool.tile([128, NT * VW], BF, tag="v_sb")
            vf_sb = cpool.tile([128, 128], BF, tag="vf_sb")
            vl_sb = cpool.tile([128, 128], BF, tag="vl_sb")
            u34_sb = cpool.tile([128, 2], BF, tag="u34_sb")
            r34_sb = cpool.tile([128, 8], DT, tag="r34_sb")

            # ---- DMA issue order drives queue service order ----
            # W chunks and x tiles interleaved to match the diagonal quad
            # matmul schedule, so the in-order PE queue never blocks on a
            # far-away DMA.
            def dma_x(lo, hi):
                nc.sync.dma_start(x8_sb[:, lo * 1024:hi * 1024],
                                  x8[:, lo * 1024:hi * 1024])
                nc.sync.dma_start(dx8_sb[:, lo * 1024:hi * 1024],
                                  dx8[:, lo * 1024:hi * 1024])

            def dma_w(kt2):
                nc.sync.dma_start(w8i_sb[:, kt2 * 2048:(kt2 + 1) * 2048],
                                  w8i[:, kt2 * 2048:(kt2 + 1) * 2048])
                nc.sync.dma_start(dw8i_sb[:, kt2 * 2048:(kt2 + 1) * 2048],
                                  dw8i[:, kt2 * 2048:(kt2 + 1) * 2048])

            def dma_xonly(a8, sb, lo, hi):
                nc.sync.dma_start(sb[:, lo * 1024:hi * 1024],
                                  a8[:, lo * 1024:hi * 1024])

            # pass-major mm1 (main pass first, residual passes later) lets the
            # main-weight chunks stream first and the residuals follow
            nc.sync.dma_start(w8i_sb[:, 0:2048], w8i[:, 0:2048])
            dma_xonly(x8, x8_sb, 0, 1)
            dma_xonly(x8, x8_sb, 1, 2)
            nc.sync.dma_start(w8i_sb[:, 2048:4096], w8i[:, 2048:4096])
            dma_xonly(x8, x8_sb, 2, 4)
            for kt2 in range(2, KT2):
                nc.sync.dma_start(w8i_sb[:, kt2 * 2048:(kt2 + 1) * 2048],
                                  w8i[:, kt2 * 2048:(kt2 + 1) * 2048])
            for kt2 in range(KT2):
                nc.sync.dma_start(dw8i_sb[:, kt2 * 2048:(kt2 + 1) * 2048],
                                  dw8i[:, kt2 * 2048:(kt2 + 1) * 2048])
            dma_xonly(dx8, dx8_sb, 0, 2)
            dma_xonly(dx8, dx8_sb, 2, 4)
            dma_x(4, 6)
            dma_x(6, 8)
            dma_x(8, 9)
            nc.sync.dma_start(a_sb[:], a_tab[:])
            nc.sync.dma_start(wfl_sb[:], wfl[:])
            dma_x(9, 11)
            dma_x(11, 13)
            dma_x(13, 15)
            dma_x(15, 16)
            nc.sync.dma_start(w34_sb[:], w34[:])
            nc.sync.dma_start(w8o_sb[:], w8o[:])
            nc.sync.dma_start(dw8o_sb[:], dw8o[:])

            def _mm1_chunks(i):
                if i == 0:
                    return ((0, 512), (640, 384))
                if i == NT - 1:
                    return ((0, 384), (512, 512))
                return ((0, 384), (640, 384))

            def _mm1_copies(i, pv):
                # banded head cols -> v_sb (scaled units, bf16), alternating
                # DVE/Act per tile so neither engine backlogs and PSUM slots
                # recycle at the PE rate.
                if i == NT - 1:
                    # slab-1 head 0 and u4 both wait on these: vl first (for
                    # u4), then the v halves split across DVE+Act
                    nc.vector.tensor_copy(vl_sb[:], pv[:, 512:640])
                    nc.scalar.copy(v_sb[:, i * VW:i * VW + 384], pv[:, 0:384])
                    nc.vector.tensor_copy(
                        v_sb[:, i * VW + 384:(i + 1) * VW], pv[:, 640:1024]
                    )
                    return
                if i % 2 == 1:
                    cp = nc.vector.tensor_copy
                else:
                    cp = nc.scalar.copy
                cp(v_sb[:, i * VW:i * VW + 384], pv[:, 0:384])
                cp(v_sb[:, i * VW + 384:(i + 1) * VW], pv[:, 640:1024])
                if i == 0:
                    nc.vector.tensor_copy(vf_sb[:], pv[:, 384:512])

            def _mm1_mms(pv, i, pass_major):
                passes = [(0, x8_sb, w8i_sb), (1, dx8_sb, w8i_sb),
                          (2, x8_sb, dw8i_sb)]
                if pass_major:
                    # dx pass last: its DMA tiles arrive after the x stream
                    passes = [passes[0], passes[2], passes[1]]
                    order = [(p, kt2) for p in range(3) for kt2 in range(KT2)]
                else:
                    order = [(p, kt2) for kt2 in range(KT2) for p in range(3)]
                for n_, (p, kt2) in enumerate(order):
                    _, xa, wa = passes[p]
                    lhsT = xa[:, i * 1024 + kt2 * 256:
                              i * 1024 + (kt2 + 1) * 256]
                    lhsT = lhsT.rearrange("p (s l) -> p s l", s=2)
                    wview = wa[:, kt2 * 2048:(kt2 + 1) * 2048]
                    wview = wview.rearrange("p (s m) -> p s m", s=2)
                    for m0, n in _mm1_chunks(i):
                        nc.tensor.matmul(
                            pv[:, m0:m0 + n],
                            lhsT,
                            wview[:, :, m0:m0 + n],
                            start=(n_ == 0),
                            stop=(n_ == len(order) - 1),
                            perf_mode=DR,
                        )

            def mm1_quad(ps_v):
                """Tiles 0-3 interleaved, pass-major (main, W-residual,
                x-residual) with kt2 inner, matching the DMA stream, so
                during the fill the in-order PE queue always has work.
                Tiles 2-3 borrow the (idle until attention) ps_att slots."""
                pvs = [
                    ps_v.tile([128, 1024], DT, tag="pv", name="pv0"),
                    ps_v.tile([128, 1024], DT, tag="pv", name="pv1"),
                    ps_att.tile([128, 1024], DT, tag="patt", name="pv2"),
                    ps_att.tile([128, 1024], DT, tag="patt", name="pv3"),
                ]
                passes = ((x8_sb, w8i_sb), (x8_sb, dw8i_sb), (dx8_sb, w8i_sb))
                for p, (xa, wa) in enumerate(passes):
                    for kt2 in range(KT2):
                        for i in range(4):
                            lhsT = xa[:, i * 1024 + kt2 * 256:
                                      i * 1024 + (kt2 + 1) * 256]
                            lhsT = lhsT.rearrange("p (s l) -> p s l", s=2)
                            wview = wa[:, kt2 * 2048:(kt2 + 1) * 2048]
                            wview = wview.rearrange("p (s m) -> p s m", s=2)
                            for m0, n in _mm1_chunks(i):
                                nc.tensor.matmul(
                                    pvs[i][:, m0:m0 + n],
                                    lhsT,
                                    wview[:, :, m0:m0 + n],
                                    start=(p == 0 and kt2 == 0),
                                    stop=(p == 2 and kt2 == KT2 - 1),
                                    perf_mode=DR,
                                )
                for i in range(4):
                    _mm1_copies(i, pvs[i])

            def mm1_tile(ps_v, i):
                pv = ps_v.tile([128, 1024], DT, tag="pv")
                _mm1_mms(pv, i, pass_major=True)
                _mm1_copies(i, pv)

            def attn_head(s, bi, att_hi, att_lo):
                """Banded attention for q-slab s, head bi: accumulate banded
                bf16 matmul pieces into patt (= att*SA), then split to fp8
                hi (Act copy) + lo residual (tensor_sub halves on DVE+Pool)."""
                t = BTYPE[bi]
                mms = []
                for i in range(NT):
                    for q0, q1, c0, base in _att_pieces(i):
                        if not (1024 * s <= q0 < 1024 * (s + 1)):
                            continue
                        mms.append((q0, q1, c0, base, i, (q0 - 1024 * s) // 512))
                # the slab's last-produced v tile goes last, so the head's
                # first matmuls don't wait on that tile's v copy
                lastv = 8 if s == 0 else NT - 1
                mms.sort(key=lambda mm: mm[4] == lastv)
                last_of_bank = {}
                for n_, mm in enumerate(mms):
                    last_of_bank[mm[5]] = n_
                patt = ps_att.tile([128, 1024], DT, tag="patt")
                started = set()
                for n_, (q0, q1, c0, base, i, bank) in enumerate(mms):
                    first = bank not in started
                    started.add(bank)
                    col = t * AT_STRIDE + base + c0
                    nc.tensor.matmul(
                        patt[:, q0 - 1024 * s:q1 - 1024 * s],
                        v_sb[:, i * VW + bi * 128:i * VW + (bi + 1) * 128],
                        a_sb[:, col:col + (q1 - q0)],
                        start=first,
                        stop=(last_of_bank[bank] == n_),
                    )
                c0 = bi * 1024
                nc.scalar.copy(att_hi[:, c0:c0 + 1024], patt[:])
                # residual on DVE (GPSIMD cannot read PSUM on hardware); the
                # head interleaving gives the copy chain room to drain
                nc.vector.tensor_sub(
                    att_lo[:, c0:c0 + 1024], patt[:], att_hi[:, c0:c0 + 1024]
                )

            att_hi = [None, None]
            att_lo = [None, None]

            # ---- rank-1 'first'/'last' head correction pieces ----
            def mk_u(pool, tag, col, vsrc):
                # u = wfl-col @ v-tile  [128 m, 1], kept in scaled units
                pu = pool.tile([128, 1024], DT, tag=tag, name=f"pu{col}")
                nc.tensor.matmul(pu[:, 0:1], vsrc[:], wfl_sb[:, col:col + 1],
                                 start=True, stop=True)
                nc.scalar.copy(u34_sb[:, col:col + 1], pu[:, 0:1])

            def mk_r34():
                # r34T[e] = sum_m W_outT[384+m,e] u3[m] + W_outT[512+m,e] u4[m]
                pr = ps_att.tile([128, 1024], DT, tag="patt", name="pr")
                for t in range(8):
                    for hh in range(2):
                        nc.tensor.matmul(
                            pr[:, t:t + 1],
                            w34_sb[:, hh * E + t * 128:hh * E + (t + 1) * 128],
                            u34_sb[:, hh:hh + 1],
                            start=(hh == 0),
                            stop=(hh == 1),
                        )
                nc.scalar.copy(r34_sb[:], pr[:, 0:8])

            # PE warmup: dummy matmuls on a memset tile while the first
            # DMAs land; pulls the p-state ramp forward, fills the gap.
            zw = cpool.tile([128, 512], BF, tag="zw")
            nc.vector.memset(zw[:], 0)
            for wi in range(WARM):
                pw = ps_att.tile([128, 1024], DT, tag="patt", name=f"pw{wi}")
                nc.tensor.matmul(
                    pw[:, 0:256], zw[:, 0:128], zw[:, 0:256],
                    start=True, stop=True,
                )

            def mm2_quarter(ps_o, c, trange):
                # out^T[e, q-chunk c] for e-tiles in trange: 3 fp8 DR passes
                # over 3 banded-head pairs, then a fused Act copy
                # Identity(po/65536 + r34T bias) and the output DMA.
                s = c // 2
                hi_v = att_hi[s][:].rearrange("p (bi q) -> p bi q", bi=NBH)
                lo_v = att_lo[s][:].rearrange("p (bi q) -> p bi q", bi=NBH)
                q0 = (c % 2) * 512
                for t in trange:
                    po = ps_o.tile([128, 512], DT, tag="po")
                    for p, av in enumerate((hi_v, lo_v, hi_v)):
                        wv = (w8o_sb if p < 2 else dw8o_sb)[:].rearrange(
                            "p (bi e) -> p bi e", bi=NBH
                        )
                        for hp in range(3):
                            nc.tensor.matmul(
                                po[:],
                                wv[:, 2 * hp:2 * hp + 2,
                                   t * 128:(t + 1) * 128],
                                av[:, 2 * hp:2 * hp + 2, q0:q0 + 512],
                                start=(p == 0 and hp == 0),
                                stop=(p == 2 and hp == 2),
                                perf_mode=DR,
                            )
                    ot = outpool.tile([128, 512], DT, tag="out")
                    nc.scalar.activation(
                        ot[:], po[:],
                        mybir.ActivationFunctionType.Identity,
                        bias=r34_sb[:, t:t + 1], scale=INV2,
                    )
                    nc.sync.dma_start(
                        out[t * 128:(t + 1) * 128, c * 512:(c + 1) * 512],
                        ot[:],
                    )

            with tc.tile_pool(name="ps_v", bufs=2, space="PSUM") as ps_v:
                mm1_quad(ps_v)

                # u3 issues early; waits only on the vf copy of tile 0
                mk_u(ps_att, "patt", 0, vf_sb)

                for i in range(4, 9):
                    mm1_tile(ps_v, i)

                att_hi[0] = attpool.tile([128, NBH * 1024], F8,
                                         tag="hi", name="hi0")
                att_lo[0] = attpool.tile([128, NBH * 1024], F8,
                                         tag="lo", name="lo0")
                # slab-0 heads interleaved with mm1 tiles 9-14: each head's
                # hi/lo copy chain drains during the next tile's matmuls
                for bi in range(NBH):
                    attn_head(0, bi, att_hi[0], att_lo[0])
                    mm1_tile(ps_v, 9 + bi)
                mm1_tile(ps_v, 15)

            att_hi[1] = attpool.tile([128, NBH * 1024], F8,
                                     tag="hi", name="hi1")
            att_lo[1] = attpool.tile([128, NBH * 1024], F8,
                                     tag="lo", name="lo1")
            attn_head(1, 0, att_hi[1], att_lo[1])
            mk_u(ps_att, "patt", 1, vl_sb)
            attn_head(1, 1, att_hi[1], att_lo[1])
            mk_r34()

            # ---- mm2 interleaved with the remaining slab-1 heads ----
            with tc.tile_pool(name="ps_o", bufs=4, space="PSUM") as ps_o:
                attn_head(1, 2, att_hi[1], att_lo[1])
                mm2_quarter(ps_o, 0, range(0, 4))
                attn_head(1, 3, att_hi[1], att_lo[1])
                mm2_quarter(ps_o, 0, range(4, 8))
                attn_head(1, 4, att_hi[1], att_lo[1])
                mm2_quarter(ps_o, 1, range(0, 4))
                attn_head(1, 5, att_hi[1], att_lo[1])
                mm2_quarter(ps_o, 1, range(4, 8))
                mm2_quarter(ps_o, 2, range(0, 8))
                mm2_quarter(ps_o, 3, range(0, 8))

    nc.compile()
    return nc


class _Runner:
    """Builds the Bass program once and caches a jitted shard_map executable
    (one batch element per NeuronCore)."""

    IN_ORDER = ["x8", "dx8", "w8i", "dw8i", "w8o", "dw8o", "a_tab", "w34", "wfl"]

    def __init__(self):
        import jax
        from jax.sharding import Mesh, PartitionSpec
        from jax.experimental.shard_map import shard_map

        self.jax = jax
        _b2j.install_neuronx_cc_hook()
        nc = _build_program()
        self.nc = nc
        self.a_tab_np, self.wfl_np = _host_tables()

        partition_name = (
            nc.partition_id_tensor.name if nc.partition_id_tensor else None
        )
        in_names = []
        out_names = []
        out_avals = []
        for alloc in nc.m.functions[0].allocations:
            if not isinstance(alloc, mybir.MemoryLocationSet):
                continue
            name = alloc.memorylocations[0].name
            if alloc.kind == "ExternalInput":
                if name != partition_name:
                    in_names.append(name)
            elif alloc.kind == "ExternalOutput":
                out_names.append(name)
                out_avals.append(
                    jax.core.ShapedArray(
                        tuple(alloc.tensor_shape), mybir.dt.np(alloc.dtype)
                    )
                )
        assert sorted(in_names) == sorted(self.IN_ORDER), in_names
        self.in_names = in_names
        self.out_names = out_names
        self.out_avals = out_avals
        n_params = len(in_names)
        n_outs = len(out_names)
        all_names = tuple(in_names) + tuple(out_names)
        if partition_name is not None:
            all_names = all_names + (partition_name,)

        def _body(*args):
            operands = list(args)
            if partition_name is not None:
                operands.append(_b2j.partition_id_tensor())
            outs = _b2j._bass_exec_p.bind(
                *operands,
                out_avals=tuple(out_avals),
                in_names=all_names,
                out_names=tuple(out_names),
                lowering_input_output_aliases=(),
                sim_require_finite=True,
                sim_require_nnan=True,
                nc=nc,
            )
            return tuple(outs)

        devices = jax.devices()[:B]
        assert len(devices) == B
        self.mesh = Mesh(np.asarray(devices), ("core",))
        in_specs = (PartitionSpec("core"),) * (n_params + n_outs)
        out_specs = (PartitionSpec("core"),) * n_outs
        self.sharded = jax.jit(
            shard_map(
                _body,
                mesh=self.mesh,
                in_specs=in_specs,
                out_specs=out_specs,
                check_rep=False,
            ),
            donate_argnums=tuple(range(n_params, n_params + n_outs)),
            keep_unused=True,
        )

    def _concat_static(self, statics):
        jax = self.jax
        out = {}
        for name, arr in statics.items():
            big = np.concatenate([arr] * B, axis=0)
            out[name] = jax.device_put(big)
        return out

    def run_device(self, dev_args):
        jnp = self.jax.numpy
        zeros = [
            jnp.zeros((B * av.shape[0], *av.shape[1:]), av.dtype)
            for av in self.out_avals
        ]
        return self.sharded(*dev_args, *zeros)

    def prepare_inputs(self, x, W_in, W_out):
        # ---- x: per batch, 2-level e4m3 at scale SX, DoubleRow layout ----
        xs = x.reshape(B, L, E) * np.float32(SX)
        x8 = xs.astype(NPF8)
        dx8 = (xs - x8.astype(np.float32)).astype(NPF8)

        def dr_x(a8):  # [B, L, E] fp8 -> [B*128, NT*1024]
            t = a8.reshape(B, NT, 128, KT2, 2, 128)   # b, i, l, kt2, s, p
            t = t.transpose(0, 5, 1, 3, 4, 2)         # b, p, i, kt2, s, l
            return np.ascontiguousarray(t).reshape(B * 128, NT * 1024)

        # ---- W_in.T: 2-level e4m3 at scale SWI, DoubleRow layout ----
        wiT = W_in.T * np.float32(SWI)
        w8 = wiT.astype(NPF8)
        dw8 = (wiT - w8.astype(np.float32)).astype(NPF8)

        def dr_wi(a8):  # [E, E] fp8 -> [128, KT2*2048]
            t = a8.reshape(KT2, 2, 128, E)            # kt2, s, p, m
            t = t.transpose(2, 0, 1, 3)               # p, kt2, s, m
            return np.ascontiguousarray(t).reshape(128, KT2 * 2048)

        # ---- W_out.T banded rows: 2-level e4m3 at scale SWO, pair layout ----
        woT = W_out.T * np.float32(SWO)
        wo8 = woT.astype(NPF8)
        dwo8 = (woT - wo8.astype(np.float32)).astype(NPF8)

        def dr_wo(a8):  # [E, E] fp8 -> [128, NBH*1024]
            t = np.stack([a8[h * 128:(h + 1) * 128, :] for h in BANDED_HEADS])
            t = t.transpose(1, 0, 2)                  # p, bi, e
            return np.ascontiguousarray(t).reshape(128, NBH * E)

        # ---- W_out.T rows for heads 3/4 (bf16, pre-descaled: u34 carries
        # the v-scale 65536, so fold 1/65536 here to make r34 natural) ----
        w34 = (W_out.T[384:640, :] * np.float32(INV1))
        w34 = w34.reshape(2, 128, E).transpose(1, 0, 2)
        w34 = np.ascontiguousarray(w34).reshape(128, 2 * E).astype(NPBF)

        statics = {
            "w8i": dr_wi(w8),
            "dw8i": dr_wi(dw8),
            "w8o": dr_wo(wo8),
            "dw8o": dr_wo(dwo8),
            "a_tab": self.a_tab_np,
            "w34": w34,
            "wfl": self.wfl_np,
        }
        dev = self._concat_static(statics)
        dev["x8"] = self.jax.device_put(dr_x(x8))
        dev["dx8"] = self.jax.device_put(dr_x(dx8))
        return [dev[name] for name in self.in_names]

    def __call__(self, x, W_in, W_out):
        args = self.prepare_inputs(x, W_in, W_out)
        outs = self.run_device(args)
        outT = np.asarray(outs[self.out_names.index("out")])  # [B*E, L]
        return np.ascontiguousarray(outT.reshape(B, E, L).transpose(0, 2, 1))


_CACHE = {}


def _get_runner() -> _Runner:
    if "runner" not in _CACHE:
        _CACHE["runner"] = _Runner()
    return _CACHE["runner"]


def kernel(x, W_in, W_out):
    x = np.ascontiguousarray(np.asarray(x, dtype=np.float32))
    W_in = np.ascontiguousarray(np.asarray(W_in, dtype=np.float32))
    W_out = np.ascontiguousarray(np.asarray(W_out, dtype=np.float32))
    assert x.shape == (B, L, E)
    return _get_runner()(x, W_in, W_out)


if __name__ == "__main__":
    rng = np.random.default_rng(0)
    x = rng.standard_normal((B, L, E), dtype=np.float32)
    W_in = rng.standard_normal((E, E), dtype=np.float32) * 0.05
    W_out = rng.standard_normal((E, E), dtype=np.float32) * 0.05
    y = kernel(x, W_in, W_out)
    print("out", y.shape, y.dtype, np.abs(y).mean())


# revision 61
# speedup vs baseline: 1.4774x; 1.0158x over previous
"""Trainium2 Bass kernel for nn_NewAttention (analytic Gaussian sparse attention).

Math (per batch element b):
    v        = x[b] @ W_in.T                      # [L, E]
    per head h (P=128 cols of v):
        A_h  = softmax(-(j - c_h(i))^2 / 2)       # [L, L], analytic, banded
        att_h = A_h @ v_h                         # [L, P]
    out[b]   = concat_h(att_h) @ W_out.T          # [L, E]

Sharding: data-parallel over batch, one batch element per NeuronCore (8 cores).

Device strategy (per core):
  - mm1 (v = x @ W_in.T) runs as 3 fp8-e4m3 DoubleRow passes (main +
    x-residual + W-residual; the dropped cross term is ~1e-3 relative).
    Host pre-quantizes x*32 and W_in.T*2048 plus their e4m3 residuals and
    lays both out in the DoubleRow [128, 2, .] pairing. PSUM accumulates all
    3 passes; a DVE copy scales by 1/65536 into bf16 v_sb (natural units).
  - attention: att^T_h = v_h.T @ (A^T * 32) as banded bf16 matmuls with
    144-wide analytic weight blocks (one interior block per head type plus
    boundary-renormalized first/last blocks; band halo +-7, truncation
    ~1e-14). PSUM accumulates overlapping windows via has_written bits.
    patt (= att*32) is copied twice: Act -> fp8 att_hi, DVE tensor_sub ->
    fp8 att_lo residual.
  - mm2 produces out^T (partition = embed dim): 3 fp8 DoubleRow passes
    (hi@W8o + lo@W8o + hi@dW8o) over 3 banded-head pairs. The PSUM->SBUF
    copy is a fused Activation Identity(po*1/65536 + r34T bias), where the
    per-partition bias vector r34T carries the rank-1 'first'/'last' head
    contribution (they attend to a fixed key location for every query).
  - output DMA'd as out^T [E, L]; host transposes back.
"""

import sys
import numpy as np

for _p in ("/opt/trn_rl_repo",):
    if _p not in sys.path:
        sys.path.insert(0, _p)

import concourse.bass as bass
import concourse.bacc as bacc
import concourse.mybir as mybir
from concourse import tile
from concourse import bass2jax as _b2j
import ml_dtypes

# ---------------- problem constants (hardcoded per contract) ----------------
B = 8
L = 2048
E = 1024
H = 8
P = 128
SIGMA = 1.0
DISP = 1
NT = L // 128           # 16 l-tiles
KT2 = E // 256          # 4 DoubleRow contraction groups
DT = mybir.dt.float32
BF = mybir.dt.bfloat16
F8 = mybir.dt.float8e4
DR = mybir.MatmulPerfMode.DoubleRow
NPF8 = ml_dtypes.float8_e4m3
NPBF = ml_dtypes.bfloat16

SX = 32.0        # x fp8 scale
SWI = 2048.0     # W_in fp8 scale
SA = 32.0        # att fp8 scale (folded into A table)
SWO = 2048.0     # W_out fp8 scale
INV1 = 1.0 / (SX * SWI)
INV2 = 1.0 / (SA * SWO)

WARM = 16        # PE warmup matmuls during initial DMA fill

BANDED_HEADS = [0, 1, 2, 5, 6, 7]   # center,left,right,center,left,right
NBH = len(BANDED_HEADS)
BTYPE = [0, 1, 2, 0, 1, 2]          # per banded idx: 0=center,1=left,2=right
TYPE_DISP = [0, -DISP, +DISP]
VW = NBH * 128                      # 768 banded v cols per tile
# A-table layout per type: [interior 144 | first 136 | last 136]
AT_INT, AT_FIRST, AT_LAST, AT_STRIDE = 0, 144, 280, 416


def _att_pieces(i):
    """Attention pieces for v-tile i: (q0, q1, block_col0, block_base_off).

    Window of q positions tile i contributes to (halo +-7 around the tile,
    clipped at sequence bounds), split at PSUM 512-col bank boundaries.
    """
    if i == 0:
        w0, wid, base = 0, 136, AT_FIRST
    elif i == NT - 1:
        w0, wid, base = 128 * i - 8, 136, AT_LAST
    else:
        w0, wid, base = 128 * i - 8, 144, AT_INT
    out = []
    q = w0
    while q < w0 + wid:
        qe = min(w0 + wid, (q // 512 + 1) * 512)
        out.append((q, qe, q - w0, base))
        q = qe
    return out


def _softmax_rows(logits):
    m = logits.max(axis=-1, keepdims=True)
    e = np.exp(logits - m)
    return e / e.sum(axis=-1, keepdims=True)


def _host_tables():
    """Analytic attention weight blocks (exact, float64 -> bf16, scaled by SA)
    and the first/last-head key-weight vectors."""
    j = np.arange(L, dtype=np.float64)
    i = np.arange(L, dtype=np.float64)

    a_tab = np.zeros((128, 3 * AT_STRIDE), dtype=np.float64)
    for t, disp in enumerate(TYPE_DISP):
        c = i + disp
        logits = -((j[None, :] - c[:, None]) ** 2) / (2.0 * SIGMA**2)
        A = _softmax_rows(logits)  # [Lq, Lk]
        base = t * AT_STRIDE
        # interior block from representative tile 4: B[p, c] = A[504+c, 512+p]
        a_tab[:, base + AT_INT:base + AT_INT + 144] = A[504:648, 512:640].T
        a_tab[:, base + AT_FIRST:base + AT_FIRST + 136] = A[0:136, 0:128].T
        a_tab[:, base + AT_LAST:base + AT_LAST + 136] = A[1912:2048, 1920:2048].T
    a_tab *= SA

    Af = _softmax_rows(-((j[None, :]) ** 2) / (2 * SIGMA**2))
    Al = _softmax_rows(-((j[None, :] - (L - 1.0)) ** 2) / (2 * SIGMA**2))
    wfl = np.zeros((128, 2), dtype=np.float64)
    wfl[:, 0] = Af[0, 0:128]         # 'first': support at k < 128 (v tile 0)
    wfl[:, 1] = Al[0, L - 128:L]     # 'last': support at k >= L-128 (tile 15)

    # v_sb is held in scaled units (v * SX*SWI); fold the descale into the
    # attention table so the v copy is a plain TensorCopy.
    a_tab *= INV1
    return a_tab.astype(NPBF), wfl.astype(NPBF)


def _build_program():
    nc = bacc.Bacc("TRN2", target_bir_lowering=False, debug=False, num_devices=B)

    x8 = nc.dram_tensor("x8", [128, NT * 1024], F8, kind="ExternalInput")
    dx8 = nc.dram_tensor("dx8", [128, NT * 1024], F8, kind="ExternalInput")
    w8i = nc.dram_tensor("w8i", [128, KT2 * 2048], F8, kind="ExternalInput")
    dw8i = nc.dram_tensor("dw8i", [128, KT2 * 2048], F8, kind="ExternalInput")
    w8o = nc.dram_tensor("w8o", [128, NBH * 1024], F8, kind="ExternalInput")
    dw8o = nc.dram_tensor("dw8o", [128, NBH * 1024], F8, kind="ExternalInput")
    a_tab = nc.dram_tensor("a_tab", [128, 3 * AT_STRIDE], BF, kind="ExternalInput")
    w34 = nc.dram_tensor("w34", [128, 2 * E], BF, kind="ExternalInput")
    wfl = nc.dram_tensor("wfl", [128, 2], BF, kind="ExternalInput")
    out = nc.dram_tensor("out", [E, L], DT, kind="ExternalOutput")

    with tile.TileContext(nc) as tc:
        with (
            tc.tile_pool(name="const", bufs=1) as cpool,
            tc.tile_pool(name="vbuf", bufs=1) as vpool,
            tc.tile_pool(name="attb", bufs=2) as attpool,
            tc.tile_pool(name="outp", bufs=6) as outpool,
            tc.tile_pool(name="ps_att", bufs=2, space="PSUM") as ps_att,
        ):
            x8_sb = cpool.tile([128, NT * 1024], F8, tag="x8_sb")
            dx8_sb = cpool.tile([128, NT * 1024], F8, tag="dx8_sb")
            w8i_sb = cpool.tile([128, KT2 * 2048], F8, tag="w8i_sb")
            dw8i_sb = cpool.tile([128, KT2 * 2048], F8, tag="dw8i_sb")
            w8o_sb = cpool.tile([128, NBH * 1024], F8, tag="w8o_sb")
            dw8o_sb = cpool.tile([128, NBH * 1024], F8, tag="dw8o_sb")
            a_sb = cpool.tile([128, 3 * AT_STRIDE], BF, tag="a_sb")
            w34_sb = cpool.tile([128, 2 * E], BF, tag="w34_sb")
            wfl_sb = cpool.tile([128, 2], BF, tag="wfl_sb")
            v_sb = vpool.tile([128, NT * VW], BF, tag="v_sb")
            vf_sb = cpool.tile([128, 128], BF, tag="vf_sb")
            vl_sb = cpool.tile([128, 128], BF, tag="vl_sb")
            u34_sb = cpool.tile([128, 2], BF, tag="u34_sb")
            r34_sb = cpool.tile([128, 8], DT, tag="r34_sb")

            # ---- DMA issue order drives queue service order ----
            # W chunks and x tiles interleaved to match the diagonal quad
            # matmul schedule, so the in-order PE queue never blocks on a
            # far-away DMA.
            def dma_x(lo, hi):
                nc.sync.dma_start(x8_sb[:, lo * 1024:hi * 1024],
                                  x8[:, lo * 1024:hi * 1024])
                nc.sync.dma_start(dx8_sb[:, lo * 1024:hi * 1024],
                                  dx8[:, lo * 1024:hi * 1024])

            def dma_w(kt2):
                nc.sync.dma_start(w8i_sb[:, kt2 * 2048:(kt2 + 1) * 2048],
                                  w8i[:, kt2 * 2048:(kt2 + 1) * 2048])
                nc.sync.dma_start(dw8i_sb[:, kt2 * 2048:(kt2 + 1) * 2048],
                                  dw8i[:, kt2 * 2048:(kt2 + 1) * 2048])

            def dma_xonly(a8, sb, lo, hi):
                nc.sync.dma_start(sb[:, lo * 1024:hi * 1024],
                                  a8[:, lo * 1024:hi * 1024])

            # pass-major mm1 (main pass first, residual passes later) lets the
            # main-weight chunks stream first and the residuals follow
            nc.sync.dma_start(w8i_sb[:, 0:2048], w8i[:, 0:2048])
            dma_xonly(x8, x8_sb, 0, 1)
            dma_xonly(x8, x8_sb, 1, 2)
            nc.sync.dma_start(w8i_sb[:, 2048:4096], w8i[:, 2048:4096])
            dma_xonly(x8, x8_sb, 2, 4)
            for kt2 in range(2, KT2):
                nc.sync.dma_start(w8i_sb[:, kt2 * 2048:(kt2 + 1) * 2048],
                                  w8i[:, kt2 * 2048:(kt2 + 1) * 2048])
            for kt2 in range(KT2):
                nc.sync.dma_start(dw8i_sb[:, kt2 * 2048:(kt2 + 1) * 2048],
                                  dw8i[:, kt2 * 2048:(kt2 + 1) * 2048])
            dma_xonly(dx8, dx8_sb, 0, 2)
            dma_xonly(x8, x8_sb, 4, 6)
            dma_xonly(dx8, dx8_sb, 2, 4)
            dma_xonly(x8, x8_sb, 6, 8)
            dma_xonly(dx8, dx8_sb, 4, 6)
            dma_xonly(x8, x8_sb, 8, 9)
            dma_xonly(dx8, dx8_sb, 6, 9)
            nc.sync.dma_start(a_sb[:], a_tab[:])
            nc.sync.dma_start(wfl_sb[:], wfl[:])
            dma_x(9, 11)
            dma_x(11, 13)
            dma_x(13, 15)
            dma_x(15, 16)
            nc.sync.dma_start(w34_sb[:], w34[:])
            nc.sync.dma_start(w8o_sb[:], w8o[:])
            nc.sync.dma_start(dw8o_sb[:], dw8o[:])

            def _mm1_chunks(i):
                if i == 0:
                    return ((0, 512), (640, 384))
                if i == NT - 1:
                    return ((0, 384), (512, 512))
                return ((0, 384), (640, 384))

            def _mm1_copies(i, pv):
                # banded head cols -> v_sb (scaled units, bf16), alternating
                # DVE/Act per tile so neither engine backlogs and PSUM slots
                # recycle at the PE rate.
                if i == NT - 1:
                    # slab-1 head 0 and u4 both wait on these: vl first (for
                    # u4), then the v halves split across DVE+Act
                    nc.vector.tensor_copy(vl_sb[:], pv[:, 512:640])
                    nc.scalar.copy(v_sb[:, i * VW:i * VW + 384], pv[:, 0:384])
                    nc.vector.tensor_copy(
                        v_sb[:, i * VW + 384:(i + 1) * VW], pv[:, 640:1024]
                    )
                    return
                if i % 2 == 1:
                    cp = nc.vector.tensor_copy
                else:
                    cp = nc.scalar.copy
                cp(v_sb[:, i * VW:i * VW + 384], pv[:, 0:384])
                cp(v_sb[:, i * VW + 384:(i + 1) * VW], pv[:, 640:1024])
                if i == 0:
                    nc.vector.tensor_copy(vf_sb[:], pv[:, 384:512])

            def _mm1_mms(pv, i, pass_major):
                passes = [(0, x8_sb, w8i_sb), (1, dx8_sb, w8i_sb),
                          (2, x8_sb, dw8i_sb)]
                if pass_major:
                    # dx pass last: its DMA tiles arrive after the x stream
                    passes = [passes[0], passes[2], passes[1]]
                    order = [(p, kt2) for p in range(3) for kt2 in range(KT2)]
                else:
                    order = [(p, kt2) for kt2 in range(KT2) for p in range(3)]
                for n_, (p, kt2) in enumerate(order):
                    _, xa, wa = passes[p]
                    lhsT = xa[:, i * 1024 + kt2 * 256:
                              i * 1024 + (kt2 + 1) * 256]
                    lhsT = lhsT.rearrange("p (s l) -> p s l", s=2)
                    wview = wa[:, kt2 * 2048:(kt2 + 1) * 2048]
                    wview = wview.rearrange("p (s m) -> p s m", s=2)
                    for m0, n in _mm1_chunks(i):
                        nc.tensor.matmul(
                            pv[:, m0:m0 + n],
                            lhsT,
                            wview[:, :, m0:m0 + n],
                            start=(n_ == 0),
                            stop=(n_ == len(order) - 1),
                            perf_mode=DR,
                        )

            def mm1_quad(ps_v):
                """Tiles 0-3 interleaved, pass-major (main, W-residual,
                x-residual) with kt2 inner, matching the DMA stream, so
                during the fill the in-order PE queue always has work.
                Tiles 2-3 borrow the (idle until attention) ps_att slots."""
                pvs = [
                    ps_v.tile([128, 1024], DT, tag="pv", name="pv0"),
                    ps_v.tile([128, 1024], DT, tag="pv", name="pv1"),
                    ps_att.tile([128, 1024], DT, tag="patt", name="pv2"),
                    ps_att.tile([128, 1024], DT, tag="patt", name="pv3"),
                ]
                passes = ((x8_sb, w8i_sb), (x8_sb, dw8i_sb), (dx8_sb, w8i_sb))
                for p, (xa, wa) in enumerate(passes):
                    for kt2 in range(KT2):
                        for i in range(4):
                            lhsT = xa[:, i * 1024 + kt2 * 256:
                                      i * 1024 + (kt2 + 1) * 256]
                            lhsT = lhsT.rearrange("p (s l) -> p s l", s=2)
                            wview = wa[:, kt2 * 2048:(kt2 + 1) * 2048]
                            wview = wview.rearrange("p (s m) -> p s m", s=2)
                            for m0, n in _mm1_chunks(i):
                                nc.tensor.matmul(
                                    pvs[i][:, m0:m0 + n],
                                    lhsT,
                                    wview[:, :, m0:m0 + n],
                                    start=(p == 0 and kt2 == 0),
                                    stop=(p == 2 and kt2 == KT2 - 1),
                                    perf_mode=DR,
                                )
                for i in range(4):
                    _mm1_copies(i, pvs[i])

            def mm1_tile(ps_v, i):
                pv = ps_v.tile([128, 1024], DT, tag="pv")
                _mm1_mms(pv, i, pass_major=True)
                _mm1_copies(i, pv)

            def attn_head(s, bi, att_hi, att_lo):
                """Banded attention for q-slab s, head bi: accumulate banded
                bf16 matmul pieces into patt (= att*SA), then split to fp8
                hi (Act copy) + lo residual (tensor_sub halves on DVE+Pool)."""
                t = BTYPE[bi]
                mms = []
                for i in range(NT):
                    for q0, q1, c0, base in _att_pieces(i):
                        if not (1024 * s <= q0 < 1024 * (s + 1)):
                            continue
                        mms.append((q0, q1, c0, base, i, (q0 - 1024 * s) // 512))
                # the slab's last-produced v tile goes last, so the head's
                # first matmuls don't wait on that tile's v copy
                lastv = 8 if s == 0 else NT - 1
                mms.sort(key=lambda mm: mm[4] == lastv)
                last_of_bank = {}
                for n_, mm in enumerate(mms):
                    last_of_bank[mm[5]] = n_
                patt = ps_att.tile([128, 1024], DT, tag="patt")
                started = set()
                for n_, (q0, q1, c0, base, i, bank) in enumerate(mms):
                    first = bank not in started
                    started.add(bank)
                    col = t * AT_STRIDE + base + c0
                    nc.tensor.matmul(
                        patt[:, q0 - 1024 * s:q1 - 1024 * s],
                        v_sb[:, i * VW + bi * 128:i * VW + (bi + 1) * 128],
                        a_sb[:, col:col + (q1 - q0)],
                        start=first,
                        stop=(last_of_bank[bank] == n_),
                    )
                c0 = bi * 1024
                nc.scalar.copy(att_hi[:, c0:c0 + 1024], patt[:])
                # residual on DVE (GPSIMD cannot read PSUM on hardware); the
                # head interleaving gives the copy chain room to drain
                nc.vector.tensor_sub(
                    att_lo[:, c0:c0 + 1024], patt[:], att_hi[:, c0:c0 + 1024]
                )

            att_hi = [None, None]
            att_lo = [None, None]

            # ---- rank-1 'first'/'last' head correction pieces ----
            def mk_u(pool, tag, col, vsrc):
                # u = wfl-col @ v-tile  [128 m, 1], kept in scaled units
                pu = pool.tile([128, 1024], DT, tag=tag, name=f"pu{col}")
                nc.tensor.matmul(pu[:, 0:1], vsrc[:], wfl_sb[:, col:col + 1],
                                 start=True, stop=True)
                nc.scalar.copy(u34_sb[:, col:col + 1], pu[:, 0:1])

            def mk_r34():
                # r34T[e] = sum_m W_outT[384+m,e] u3[m] + W_outT[512+m,e] u4[m]
                pr = ps_att.tile([128, 1024], DT, tag="patt", name="pr")
                for t in range(8):
                    for hh in range(2):
                        nc.tensor.matmul(
                            pr[:, t:t + 1],
                            w34_sb[:, hh * E + t * 128:hh * E + (t + 1) * 128],
                            u34_sb[:, hh:hh + 1],
                            start=(hh == 0),
                            stop=(hh == 1),
                        )
                nc.scalar.copy(r34_sb[:], pr[:, 0:8])

            # PE warmup: dummy matmuls on a memset tile while the first
            # DMAs land; pulls the p-state ramp forward, fills the gap.
            zw = cpool.tile([128, 512], BF, tag="zw")
            nc.gpsimd.memset(zw[:], 0)
            for wi in range(WARM):
                pw = ps_att.tile([128, 1024], DT, tag="patt", name=f"pw{wi}")
                nc.tensor.matmul(
                    pw[:, 0:256], zw[:, 0:128], zw[:, 0:256],
                    start=True, stop=True,
                )

            def mm2_quarter(ps_o, c, trange):
                # out^T[e, q-chunk c] for e-tiles in trange: 3 fp8 DR passes
                # over 3 banded-head pairs, then a fused Act copy
                # Identity(po/65536 + r34T bias) and the output DMA.
                s = c // 2
                hi_v = att_hi[s][:].rearrange("p (bi q) -> p bi q", bi=NBH)
                lo_v = att_lo[s][:].rearrange("p (bi q) -> p bi q", bi=NBH)
                q0 = (c % 2) * 512
                for t in trange:
                    po = ps_o.tile([128, 512], DT, tag="po")
                    for p, av in enumerate((hi_v, lo_v, hi_v)):
                        wv = (w8o_sb if p < 2 else dw8o_sb)[:].rearrange(
                            "p (bi e) -> p bi e", bi=NBH
                        )
                        for hp in range(3):
                            nc.tensor.matmul(
                                po[:],
                                wv[:, 2 * hp:2 * hp + 2,
                                   t * 128:(t + 1) * 128],
                                av[:, 2 * hp:2 * hp + 2, q0:q0 + 512],
                                start=(p == 0 and hp == 0),
                                stop=(p == 2 and hp == 2),
                                perf_mode=DR,
                            )
                    ot = outpool.tile([128, 512], DT, tag="out")
                    nc.scalar.activation(
                        ot[:], po[:],
                        mybir.ActivationFunctionType.Identity,
                        bias=r34_sb[:, t:t + 1], scale=INV2,
                    )
                    nc.sync.dma_start(
                        out[t * 128:(t + 1) * 128, c * 512:(c + 1) * 512],
                        ot[:],
                    )

            with tc.tile_pool(name="ps_v", bufs=2, space="PSUM") as ps_v:
                mm1_quad(ps_v)

                # u3 issues early; waits only on the vf copy of tile 0
                mk_u(ps_att, "patt", 0, vf_sb)

                for i in range(4, 9):
                    mm1_tile(ps_v, i)

                att_hi[0] = attpool.tile([128, NBH * 1024], F8,
                                         tag="hi", name="hi0")
                att_lo[0] = attpool.tile([128, NBH * 1024], F8,
                                         tag="lo", name="lo0")
                # slab-0 heads interleaved with mm1 tiles 9-14: each head's
                # hi/lo copy chain drains during the next tile's matmuls
                for bi in range(NBH):
                    attn_head(0, bi, att_hi[0], att_lo[0])
                    mm1_tile(ps_v, 9 + bi)
                mm1_tile(ps_v, 15)

            att_hi[1] = attpool.tile([128, NBH * 1024], F8,
                                     tag="hi", name="hi1")
            att_lo[1] = attpool.tile([128, NBH * 1024], F8,
                                     tag="lo", name="lo1")
            attn_head(1, 0, att_hi[1], att_lo[1])
            mk_u(ps_att, "patt", 1, vl_sb)
            attn_head(1, 1, att_hi[1], att_lo[1])
            mk_r34()

            # ---- mm2 interleaved with the remaining slab-1 heads ----
            with tc.tile_pool(name="ps_o", bufs=4, space="PSUM") as ps_o:
                attn_head(1, 2, att_hi[1], att_lo[1])
                mm2_quarter(ps_o, 0, range(0, 4))
                attn_head(1, 3, att_hi[1], att_lo[1])
                mm2_quarter(ps_o, 0, range(4, 8))
                attn_head(1, 4, att_hi[1], att_lo[1])
                mm2_quarter(ps_o, 1, range(0, 4))
                attn_head(1, 5, att_hi[1], att_lo[1])
                mm2_quarter(ps_o, 1, range(4, 8))
                mm2_quarter(ps_o, 2, range(0, 8))
                mm2_quarter(ps_o, 3, range(0, 8))

    nc.compile()
    return nc


class _Runner:
    """Builds the Bass program once and caches a jitted shard_map executable
    (one batch element per NeuronCore)."""

    IN_ORDER = ["x8", "dx8", "w8i", "dw8i", "w8o", "dw8o", "a_tab", "w34", "wfl"]

    def __init__(self):
        import jax
        from jax.sharding import Mesh, PartitionSpec
        from jax.experimental.shard_map import shard_map

        self.jax = jax
        _b2j.install_neuronx_cc_hook()
        nc = _build_program()
        self.nc = nc
        self.a_tab_np, self.wfl_np = _host_tables()

        partition_name = (
            nc.partition_id_tensor.name if nc.partition_id_tensor else None
        )
        in_names = []
        out_names = []
        out_avals = []
        for alloc in nc.m.functions[0].allocations:
            if not isinstance(alloc, mybir.MemoryLocationSet):
                continue
            name = alloc.memorylocations[0].name
            if alloc.kind == "ExternalInput":
                if name != partition_name:
                    in_names.append(name)
            elif alloc.kind == "ExternalOutput":
                out_names.append(name)
                out_avals.append(
                    jax.core.ShapedArray(
                        tuple(alloc.tensor_shape), mybir.dt.np(alloc.dtype)
                    )
                )
        assert sorted(in_names) == sorted(self.IN_ORDER), in_names
        self.in_names = in_names
        self.out_names = out_names
        self.out_avals = out_avals
        n_params = len(in_names)
        n_outs = len(out_names)
        all_names = tuple(in_names) + tuple(out_names)
        if partition_name is not None:
            all_names = all_names + (partition_name,)

        def _body(*args):
            operands = list(args)
            if partition_name is not None:
                operands.append(_b2j.partition_id_tensor())
            outs = _b2j._bass_exec_p.bind(
                *operands,
                out_avals=tuple(out_avals),
                in_names=all_names,
                out_names=tuple(out_names),
                lowering_input_output_aliases=(),
                sim_require_finite=True,
                sim_require_nnan=True,
                nc=nc,
            )
            return tuple(outs)

        devices = jax.devices()[:B]
        assert len(devices) == B
        self.mesh = Mesh(np.asarray(devices), ("core",))
        in_specs = (PartitionSpec("core"),) * (n_params + n_outs)
        out_specs = (PartitionSpec("core"),) * n_outs
        self.sharded = jax.jit(
            shard_map(
                _body,
                mesh=self.mesh,
                in_specs=in_specs,
                out_specs=out_specs,
                check_rep=False,
            ),
            donate_argnums=tuple(range(n_params, n_params + n_outs)),
            keep_unused=True,
        )

    def _concat_static(self, statics):
        jax = self.jax
        out = {}
        for name, arr in statics.items():
            big = np.concatenate([arr] * B, axis=0)
            out[name] = jax.device_put(big)
        return out

    def run_device(self, dev_args):
        jnp = self.jax.numpy
        zeros = [
            jnp.zeros((B * av.shape[0], *av.shape[1:]), av.dtype)
            for av in self.out_avals
        ]
        return self.sharded(*dev_args, *zeros)

    def prepare_inputs(self, x, W_in, W_out):
        # ---- x: per batch, 2-level e4m3 at scale SX, DoubleRow layout ----
        xs = x.reshape(B, L, E) * np.float32(SX)
        x8 = xs.astype(NPF8)
        dx8 = (xs - x8.astype(np.float32)).astype(NPF8)

        def dr_x(a8):  # [B, L, E] fp8 -> [B*128, NT*1024]
            t = a8.reshape(B, NT, 128, KT2, 2, 128)   # b, i, l, kt2, s, p
            t = t.transpose(0, 5, 1, 3, 4, 2)         # b, p, i, kt2, s, l
            return np.ascontiguousarray(t).reshape(B * 128, NT * 1024)

        # ---- W_in.T: 2-level e4m3 at scale SWI, DoubleRow layout ----
        wiT = W_in.T * np.float32(SWI)
        w8 = wiT.astype(NPF8)
        dw8 = (wiT - w8.astype(np.float32)).astype(NPF8)

        def dr_wi(a8):  # [E, E] fp8 -> [128, KT2*2048]
            t = a8.reshape(KT2, 2, 128, E)            # kt2, s, p, m
            t = t.transpose(2, 0, 1, 3)               # p, kt2, s, m
            return np.ascontiguousarray(t).reshape(128, KT2 * 2048)

        # ---- W_out.T banded rows: 2-level e4m3 at scale SWO, pair layout ----
        woT = W_out.T * np.float32(SWO)
        wo8 = woT.astype(NPF8)
        dwo8 = (woT - wo8.astype(np.float32)).astype(NPF8)

        def dr_wo(a8):  # [E, E] fp8 -> [128, NBH*1024]
            t = np.stack([a8[h * 128:(h + 1) * 128, :] for h in BANDED_HEADS])
            t = t.transpose(1, 0, 2)                  # p, bi, e
            return np.ascontiguousarray(t).reshape(128, NBH * E)

        # ---- W_out.T rows for heads 3/4 (bf16, pre-descaled: u34 carries
        # the v-scale 65536, so fold 1/65536 here to make r34 natural) ----
        w34 = (W_out.T[384:640, :] * np.float32(INV1))
        w34 = w34.reshape(2, 128, E).transpose(1, 0, 2)
        w34 = np.ascontiguousarray(w34).reshape(128, 2 * E).astype(NPBF)

        statics = {
            "w8i": dr_wi(w8),
            "dw8i": dr_wi(dw8),
            "w8o": dr_wo(wo8),
            "dw8o": dr_wo(dwo8),
            "a_tab": self.a_tab_np,
            "w34": w34,
            "wfl": self.wfl_np,
        }
        dev = self._concat_static(statics)
        dev["x8"] = self.jax.device_put(dr_x(x8))
        dev["dx8"] = self.jax.device_put(dr_x(dx8))
        return [dev[name] for name in self.in_names]

    def __call__(self, x, W_in, W_out):
        args = self.prepare_inputs(x, W_in, W_out)
        outs = self.run_device(args)
        outT = np.asarray(outs[self.out_names.index("out")])  # [B*E, L]
        return np.ascontiguousarray(outT.reshape(B, E, L).transpose(0, 2, 1))


_CACHE = {}


def _get_runner() -> _Runner:
    if "runner" not in _CACHE:
        _CACHE["runner"] = _Runner()
    return _CACHE["runner"]


def kernel(x, W_in, W_out):
    x = np.ascontiguousarray(np.asarray(x, dtype=np.float32))
    W_in = np.ascontiguousarray(np.asarray(W_in, dtype=np.float32))
    W_out = np.ascontiguousarray(np.asarray(W_out, dtype=np.float32))
    assert x.shape == (B, L, E)
    return _get_runner()(x, W_in, W_out)


if __name__ == "__main__":
    rng = np.random.default_rng(0)
    x = rng.standard_normal((B, L, E), dtype=np.float32)
    W_in = rng.standard_normal((E, E), dtype=np.float32) * 0.05
    W_out = rng.standard_normal((E, E), dtype=np.float32) * 0.05
    y = kernel(x, W_in, W_out)
    print("out", y.shape, y.dtype, np.abs(y).mean())


# revision 64
# speedup vs baseline: 1.4841x; 1.0045x over previous
"""Trainium2 Bass kernel for nn_NewAttention (analytic Gaussian sparse attention).

Math (per batch element b):
    v        = x[b] @ W_in.T                      # [L, E]
    per head h (P=128 cols of v):
        A_h  = softmax(-(j - c_h(i))^2 / 2)       # [L, L], analytic, banded
        att_h = A_h @ v_h                         # [L, P]
    out[b]   = concat_h(att_h) @ W_out.T          # [L, E]

Sharding: data-parallel over batch, one batch element per NeuronCore (8 cores).

Device strategy (per core):
  - mm1 (v = x @ W_in.T) runs as 3 fp8-e4m3 DoubleRow passes (main +
    x-residual + W-residual; the dropped cross term is ~1e-3 relative).
    Host pre-quantizes x*32 and W_in.T*2048 plus their e4m3 residuals and
    lays both out in the DoubleRow [128, 2, .] pairing. PSUM accumulates all
    3 passes; a DVE copy scales by 1/65536 into bf16 v_sb (natural units).
  - attention: att^T_h = v_h.T @ (A^T * 32) as banded bf16 matmuls with
    144-wide analytic weight blocks (one interior block per head type plus
    boundary-renormalized first/last blocks; band halo +-7, truncation
    ~1e-14). PSUM accumulates overlapping windows via has_written bits.
    patt (= att*32) is copied twice: Act -> fp8 att_hi, DVE tensor_sub ->
    fp8 att_lo residual.
  - mm2 produces out^T (partition = embed dim): 3 fp8 DoubleRow passes
    (hi@W8o + lo@W8o + hi@dW8o) over 3 banded-head pairs. The PSUM->SBUF
    copy is a fused Activation Identity(po*1/65536 + r34T bias), where the
    per-partition bias vector r34T carries the rank-1 'first'/'last' head
    contribution (they attend to a fixed key location for every query).
  - output DMA'd as out^T [E, L]; host transposes back.
"""

import sys
import numpy as np

for _p in ("/opt/trn_rl_repo",):
    if _p not in sys.path:
        sys.path.insert(0, _p)

import concourse.bass as bass
import concourse.bacc as bacc
import concourse.mybir as mybir
from concourse import tile
from concourse import bass2jax as _b2j
import ml_dtypes

# ---------------- problem constants (hardcoded per contract) ----------------
B = 8
L = 2048
E = 1024
H = 8
P = 128
SIGMA = 1.0
DISP = 1
NT = L // 128           # 16 l-tiles
KT2 = E // 256          # 4 DoubleRow contraction groups
DT = mybir.dt.float32
BF = mybir.dt.bfloat16
F8 = mybir.dt.float8e4
DR = mybir.MatmulPerfMode.DoubleRow
NPF8 = ml_dtypes.float8_e4m3
NPBF = ml_dtypes.bfloat16

SX = 32.0        # x fp8 scale
SWI = 2048.0     # W_in fp8 scale
SA = 32.0        # att fp8 scale (folded into A table)
SWO = 2048.0     # W_out fp8 scale
INV1 = 1.0 / (SX * SWI)
INV2 = 1.0 / (SA * SWO)

WARM = 16        # PE warmup matmuls during initial DMA fill

BANDED_HEADS = [0, 1, 2, 5, 6, 7]   # center,left,right,center,left,right
NBH = len(BANDED_HEADS)
BTYPE = [0, 1, 2, 0, 1, 2]          # per banded idx: 0=center,1=left,2=right
TYPE_DISP = [0, -DISP, +DISP]
VW = NBH * 128                      # 768 banded v cols per tile
# A-table layout per type: [interior 144 | first 136 | last 136]
AT_INT, AT_FIRST, AT_LAST, AT_STRIDE = 0, 144, 280, 416


def _att_pieces(i):
    """Attention pieces for v-tile i: (q0, q1, block_col0, block_base_off).

    Window of q positions tile i contributes to (halo +-7 around the tile,
    clipped at sequence bounds), split at PSUM 512-col bank boundaries.
    """
    if i == 0:
        w0, wid, base = 0, 136, AT_FIRST
    elif i == NT - 1:
        w0, wid, base = 128 * i - 8, 136, AT_LAST
    else:
        w0, wid, base = 128 * i - 8, 144, AT_INT
    out = []
    q = w0
    while q < w0 + wid:
        qe = min(w0 + wid, (q // 512 + 1) * 512)
        out.append((q, qe, q - w0, base))
        q = qe
    return out


def _softmax_rows(logits):
    m = logits.max(axis=-1, keepdims=True)
    e = np.exp(logits - m)
    return e / e.sum(axis=-1, keepdims=True)


def _host_tables():
    """Analytic attention weight blocks (exact, float64 -> bf16, scaled by SA)
    and the first/last-head key-weight vectors."""
    j = np.arange(L, dtype=np.float64)
    i = np.arange(L, dtype=np.float64)

    a_tab = np.zeros((128, 3 * AT_STRIDE), dtype=np.float64)
    for t, disp in enumerate(TYPE_DISP):
        c = i + disp
        logits = -((j[None, :] - c[:, None]) ** 2) / (2.0 * SIGMA**2)
        A = _softmax_rows(logits)  # [Lq, Lk]
        base = t * AT_STRIDE
        # interior block from representative tile 4: B[p, c] = A[504+c, 512+p]
        a_tab[:, base + AT_INT:base + AT_INT + 144] = A[504:648, 512:640].T
        a_tab[:, base + AT_FIRST:base + AT_FIRST + 136] = A[0:136, 0:128].T
        a_tab[:, base + AT_LAST:base + AT_LAST + 136] = A[1912:2048, 1920:2048].T
    a_tab *= SA

    Af = _softmax_rows(-((j[None, :]) ** 2) / (2 * SIGMA**2))
    Al = _softmax_rows(-((j[None, :] - (L - 1.0)) ** 2) / (2 * SIGMA**2))
    wfl = np.zeros((128, 2), dtype=np.float64)
    wfl[:, 0] = Af[0, 0:128]         # 'first': support at k < 128 (v tile 0)
    wfl[:, 1] = Al[0, L - 128:L]     # 'last': support at k >= L-128 (tile 15)

    # v_sb is held in scaled units (v * SX*SWI); fold the descale into the
    # attention table so the v copy is a plain TensorCopy.
    a_tab *= INV1
    return a_tab.astype(NPBF), wfl.astype(NPBF)


def _build_program():
    nc = bacc.Bacc("TRN2", target_bir_lowering=False, debug=False, num_devices=B)

    x8 = nc.dram_tensor("x8", [128, NT * 1024], F8, kind="ExternalInput")
    dx8 = nc.dram_tensor("dx8", [128, NT * 1024], F8, kind="ExternalInput")
    w8i = nc.dram_tensor("w8i", [128, KT2 * 2048], F8, kind="ExternalInput")
    dw8i = nc.dram_tensor("dw8i", [128, KT2 * 2048], F8, kind="ExternalInput")
    w8o = nc.dram_tensor("w8o", [128, NBH * 1024], F8, kind="ExternalInput")
    dw8o = nc.dram_tensor("dw8o", [128, NBH * 1024], F8, kind="ExternalInput")
    a_tab = nc.dram_tensor("a_tab", [128, 3 * AT_STRIDE], BF, kind="ExternalInput")
    w34 = nc.dram_tensor("w34", [128, 2 * E], BF, kind="ExternalInput")
    wfl = nc.dram_tensor("wfl", [128, 2], BF, kind="ExternalInput")
    # bf16 output (upcast on host): halves the output DMA; the added
    # ~0.2% rounding is far inside the 2e-2 gate
    out = nc.dram_tensor("out", [E, L], BF, kind="ExternalOutput")

    with tile.TileContext(nc) as tc:
        with (
            tc.tile_pool(name="const", bufs=1) as cpool,
            tc.tile_pool(name="vbuf", bufs=1) as vpool,
            tc.tile_pool(name="attb", bufs=2) as attpool,
            tc.tile_pool(name="outp", bufs=6) as outpool,
            tc.tile_pool(name="ps_att", bufs=2, space="PSUM") as ps_att,
        ):
            x8_sb = cpool.tile([128, NT * 1024], F8, tag="x8_sb")
            dx8_sb = cpool.tile([128, NT * 1024], F8, tag="dx8_sb")
            w8i_sb = cpool.tile([128, KT2 * 2048], F8, tag="w8i_sb")
            dw8i_sb = cpool.tile([128, KT2 * 2048], F8, tag="dw8i_sb")
            w8o_sb = cpool.tile([128, NBH * 1024], F8, tag="w8o_sb")
            dw8o_sb = cpool.tile([128, NBH * 1024], F8, tag="dw8o_sb")
            a_sb = cpool.tile([128, 3 * AT_STRIDE], BF, tag="a_sb")
            w34_sb = cpool.tile([128, 2 * E], BF, tag="w34_sb")
            wfl_sb = cpool.tile([128, 2], BF, tag="wfl_sb")
            v_sb = vpool.tile([128, NT * VW], BF, tag="v_sb")
            vf_sb = cpool.tile([128, 128], BF, tag="vf_sb")
            vl_sb = cpool.tile([128, 128], BF, tag="vl_sb")
            u34_sb = cpool.tile([128, 2], BF, tag="u34_sb")
            r34_sb = cpool.tile([128, 8], DT, tag="r34_sb")

            # ---- DMA issue order drives queue service order ----
            # W chunks and x tiles interleaved to match the diagonal quad
            # matmul schedule, so the in-order PE queue never blocks on a
            # far-away DMA.
            def dma_x(lo, hi):
                nc.sync.dma_start(x8_sb[:, lo * 1024:hi * 1024],
                                  x8[:, lo * 1024:hi * 1024])
                nc.sync.dma_start(dx8_sb[:, lo * 1024:hi * 1024],
                                  dx8[:, lo * 1024:hi * 1024])

            def dma_w(kt2):
                nc.sync.dma_start(w8i_sb[:, kt2 * 2048:(kt2 + 1) * 2048],
                                  w8i[:, kt2 * 2048:(kt2 + 1) * 2048])
                nc.sync.dma_start(dw8i_sb[:, kt2 * 2048:(kt2 + 1) * 2048],
                                  dw8i[:, kt2 * 2048:(kt2 + 1) * 2048])

            def dma_xonly(a8, sb, lo, hi):
                nc.sync.dma_start(sb[:, lo * 1024:hi * 1024],
                                  a8[:, lo * 1024:hi * 1024])

            # pass-major mm1 (main pass first, residual passes later) lets the
            # main-weight chunks stream first and the residuals follow
            nc.sync.dma_start(w8i_sb[:, 0:2048], w8i[:, 0:2048])
            dma_xonly(x8, x8_sb, 0, 1)
            dma_xonly(x8, x8_sb, 1, 2)
            nc.sync.dma_start(w8i_sb[:, 2048:4096], w8i[:, 2048:4096])
            dma_xonly(x8, x8_sb, 2, 4)
            for kt2 in range(2, KT2):
                nc.sync.dma_start(w8i_sb[:, kt2 * 2048:(kt2 + 1) * 2048],
                                  w8i[:, kt2 * 2048:(kt2 + 1) * 2048])
            for kt2 in range(KT2):
                nc.sync.dma_start(dw8i_sb[:, kt2 * 2048:(kt2 + 1) * 2048],
                                  dw8i[:, kt2 * 2048:(kt2 + 1) * 2048])
            dma_xonly(dx8, dx8_sb, 0, 2)
            dma_xonly(x8, x8_sb, 4, 6)
            dma_xonly(dx8, dx8_sb, 2, 4)
            dma_xonly(x8, x8_sb, 6, 8)
            dma_xonly(dx8, dx8_sb, 4, 6)
            dma_xonly(x8, x8_sb, 8, 9)
            dma_xonly(dx8, dx8_sb, 6, 9)
            nc.sync.dma_start(a_sb[:], a_tab[:])
            nc.sync.dma_start(wfl_sb[:], wfl[:])
            dma_x(9, 11)
            dma_x(11, 13)
            dma_x(13, 15)
            dma_x(15, 16)
            nc.sync.dma_start(w34_sb[:], w34[:])
            nc.sync.dma_start(w8o_sb[:], w8o[:])
            nc.sync.dma_start(dw8o_sb[:], dw8o[:])

            def _mm1_chunks(i):
                if i == 0:
                    return ((0, 512), (640, 384))
                if i == NT - 1:
                    return ((0, 384), (512, 512))
                return ((0, 384), (640, 384))

            def _mm1_copies(i, pv):
                # banded head cols -> v_sb (scaled units, bf16), alternating
                # DVE/Act per tile so neither engine backlogs and PSUM slots
                # recycle at the PE rate.
                if i == NT - 1:
                    # slab-1 head 0 and u4 both wait on these: vl first (for
                    # u4), then the v halves split across DVE+Act
                    nc.vector.tensor_copy(vl_sb[:], pv[:, 512:640])
                    nc.scalar.copy(v_sb[:, i * VW:i * VW + 384], pv[:, 0:384])
                    nc.vector.tensor_copy(
                        v_sb[:, i * VW + 384:(i + 1) * VW], pv[:, 640:1024]
                    )
                    return
                if i % 2 == 1:
                    cp = nc.vector.tensor_copy
                else:
                    cp = nc.scalar.copy
                cp(v_sb[:, i * VW:i * VW + 384], pv[:, 0:384])
                cp(v_sb[:, i * VW + 384:(i + 1) * VW], pv[:, 640:1024])
                if i == 0:
                    nc.vector.tensor_copy(vf_sb[:], pv[:, 384:512])

            def _mm1_mms(pv, i, pass_major):
                passes = [(0, x8_sb, w8i_sb), (1, dx8_sb, w8i_sb),
                          (2, x8_sb, dw8i_sb)]
                if pass_major:
                    # dx pass last: its DMA tiles arrive after the x stream
                    passes = [passes[0], passes[2], passes[1]]
                    order = [(p, kt2) for p in range(3) for kt2 in range(KT2)]
                else:
                    order = [(p, kt2) for kt2 in range(KT2) for p in range(3)]
                for n_, (p, kt2) in enumerate(order):
                    _, xa, wa = passes[p]
                    lhsT = xa[:, i * 1024 + kt2 * 256:
                              i * 1024 + (kt2 + 1) * 256]
                    lhsT = lhsT.rearrange("p (s l) -> p s l", s=2)
                    wview = wa[:, kt2 * 2048:(kt2 + 1) * 2048]
                    wview = wview.rearrange("p (s m) -> p s m", s=2)
                    for m0, n in _mm1_chunks(i):
                        nc.tensor.matmul(
                            pv[:, m0:m0 + n],
                            lhsT,
                            wview[:, :, m0:m0 + n],
                            start=(n_ == 0),
                            stop=(n_ == len(order) - 1),
                            perf_mode=DR,
                        )

            def mm1_quad(ps_v):
                """Tiles 0-3 interleaved, pass-major (main, W-residual,
                x-residual) with kt2 inner, matching the DMA stream, so
                during the fill the in-order PE queue always has work.
                Tiles 2-3 borrow the (idle until attention) ps_att slots."""
                pvs = [
                    ps_v.tile([128, 1024], DT, tag="pv", name="pv0"),
                    ps_v.tile([128, 1024], DT, tag="pv", name="pv1"),
                    ps_att.tile([128, 1024], DT, tag="patt", name="pv2"),
                    ps_att.tile([128, 1024], DT, tag="patt", name="pv3"),
                ]
                passes = ((x8_sb, w8i_sb), (x8_sb, dw8i_sb), (dx8_sb, w8i_sb))
                for p, (xa, wa) in enumerate(passes):
                    for kt2 in range(KT2):
                        for i in range(4):
                            lhsT = xa[:, i * 1024 + kt2 * 256:
                                      i * 1024 + (kt2 + 1) * 256]
                            lhsT = lhsT.rearrange("p (s l) -> p s l", s=2)
                            wview = wa[:, kt2 * 2048:(kt2 + 1) * 2048]
                            wview = wview.rearrange("p (s m) -> p s m", s=2)
                            for m0, n in _mm1_chunks(i):
                                nc.tensor.matmul(
                                    pvs[i][:, m0:m0 + n],
                                    lhsT,
                                    wview[:, :, m0:m0 + n],
                                    start=(p == 0 and kt2 == 0),
                                    stop=(p == 2 and kt2 == KT2 - 1),
                                    perf_mode=DR,
                                )
                for i in range(4):
                    _mm1_copies(i, pvs[i])

            def mm1_tile(ps_v, i):
                pv = ps_v.tile([128, 1024], DT, tag="pv")
                _mm1_mms(pv, i, pass_major=True)
                _mm1_copies(i, pv)

            def attn_head(s, bi, att_hi, att_lo):
                """Banded attention for q-slab s, head bi: accumulate banded
                bf16 matmul pieces into patt (= att*SA), then split to fp8
                hi (Act copy) + lo residual (tensor_sub halves on DVE+Pool)."""
                t = BTYPE[bi]
                mms = []
                for i in range(NT):
                    for q0, q1, c0, base in _att_pieces(i):
                        if not (1024 * s <= q0 < 1024 * (s + 1)):
                            continue
                        mms.append((q0, q1, c0, base, i, (q0 - 1024 * s) // 512))
                # the slab's last-produced v tile goes last, so the head's
                # first matmuls don't wait on that tile's v copy
                lastv = 8 if s == 0 else NT - 1
                mms.sort(key=lambda mm: mm[4] == lastv)
                last_of_bank = {}
                for n_, mm in enumerate(mms):
                    last_of_bank[mm[5]] = n_
                patt = ps_att.tile([128, 1024], DT, tag="patt")
                started = set()
                for n_, (q0, q1, c0, base, i, bank) in enumerate(mms):
                    first = bank not in started
                    started.add(bank)
                    col = t * AT_STRIDE + base + c0
                    nc.tensor.matmul(
                        patt[:, q0 - 1024 * s:q1 - 1024 * s],
                        v_sb[:, i * VW + bi * 128:i * VW + (bi + 1) * 128],
                        a_sb[:, col:col + (q1 - q0)],
                        start=first,
                        stop=(last_of_bank[bank] == n_),
                    )
                c0 = bi * 1024
                nc.scalar.copy(att_hi[:, c0:c0 + 1024], patt[:])
                # residual on DVE (GPSIMD cannot read PSUM on hardware); the
                # head interleaving gives the copy chain room to drain
                nc.vector.tensor_sub(
                    att_lo[:, c0:c0 + 1024], patt[:], att_hi[:, c0:c0 + 1024]
                )

            att_hi = [None, None]
            att_lo = [None, None]

            # ---- rank-1 'first'/'last' head correction pieces ----
            def mk_u(pool, tag, col, vsrc):
                # u = wfl-col @ v-tile  [128 m, 1], kept in scaled units
                pu = pool.tile([128, 1024], DT, tag=tag, name=f"pu{col}")
                nc.tensor.matmul(pu[:, 0:1], vsrc[:], wfl_sb[:, col:col + 1],
                                 start=True, stop=True)
                nc.scalar.copy(u34_sb[:, col:col + 1], pu[:, 0:1])

            def mk_r34():
                # r34T[e] = sum_m W_outT[384+m,e] u3[m] + W_outT[512+m,e] u4[m]
                pr = ps_att.tile([128, 1024], DT, tag="patt", name="pr")
                for t in range(8):
                    for hh in range(2):
                        nc.tensor.matmul(
                            pr[:, t:t + 1],
                            w34_sb[:, hh * E + t * 128:hh * E + (t + 1) * 128],
                            u34_sb[:, hh:hh + 1],
                            start=(hh == 0),
                            stop=(hh == 1),
                        )
                nc.scalar.copy(r34_sb[:], pr[:, 0:8])

            # PE warmup: dummy matmuls on a memset tile while the first
            # DMAs land; pulls the p-state ramp forward, fills the gap.
            zw = cpool.tile([128, 512], BF, tag="zw")
            nc.gpsimd.memset(zw[:], 0)
            for wi in range(WARM):
                pw = ps_att.tile([128, 1024], DT, tag="patt", name=f"pw{wi}")
                nc.tensor.matmul(
                    pw[:, 0:256], zw[:, 0:128], zw[:, 0:256],
                    start=True, stop=True,
                )

            def mm2_quarter(ps_o, c, trange):
                # out^T[e, q-chunk c] for e-tiles in trange: 3 fp8 DR passes
                # over 3 banded-head pairs, then a fused Act copy
                # Identity(po/65536 + r34T bias) and the output DMA.
                s = c // 2
                hi_v = att_hi[s][:].rearrange("p (bi q) -> p bi q", bi=NBH)
                lo_v = att_lo[s][:].rearrange("p (bi q) -> p bi q", bi=NBH)
                q0 = (c % 2) * 512
                for t in trange:
                    po = ps_o.tile([128, 512], DT, tag="po")
                    for p, av in enumerate((hi_v, lo_v, hi_v)):
                        wv = (w8o_sb if p < 2 else dw8o_sb)[:].rearrange(
                            "p (bi e) -> p bi e", bi=NBH
                        )
                        for hp in range(3):
                            nc.tensor.matmul(
                                po[:],
                                wv[:, 2 * hp:2 * hp + 2,
                                   t * 128:(t + 1) * 128],
                                av[:, 2 * hp:2 * hp + 2, q0:q0 + 512],
                                start=(p == 0 and hp == 0),
                                stop=(p == 2 and hp == 2),
                                perf_mode=DR,
                            )
                    ot = outpool.tile([128, 512], BF, tag="out")
                    nc.scalar.activation(
                        ot[:], po[:],
                        mybir.ActivationFunctionType.Identity,
                        bias=r34_sb[:, t:t + 1], scale=INV2,
                    )
                    nc.sync.dma_start(
                        out[t * 128:(t + 1) * 128, c * 512:(c + 1) * 512],
                        ot[:],
                    )

            with tc.tile_pool(name="ps_v", bufs=2, space="PSUM") as ps_v:
                mm1_quad(ps_v)

                # u3 issues early; waits only on the vf copy of tile 0
                mk_u(ps_att, "patt", 0, vf_sb)

                for i in range(4, 9):
                    mm1_tile(ps_v, i)

                att_hi[0] = attpool.tile([128, NBH * 1024], F8,
                                         tag="hi", name="hi0")
                att_lo[0] = attpool.tile([128, NBH * 1024], F8,
                                         tag="lo", name="lo0")
                # slab-0 heads interleaved with mm1 tiles 9-14: each head's
                # hi/lo copy chain drains during the next tile's matmuls
                for bi in range(NBH):
                    attn_head(0, bi, att_hi[0], att_lo[0])
                    mm1_tile(ps_v, 9 + bi)
                mm1_tile(ps_v, 15)

            att_hi[1] = attpool.tile([128, NBH * 1024], F8,
                                     tag="hi", name="hi1")
            att_lo[1] = attpool.tile([128, NBH * 1024], F8,
                                     tag="lo", name="lo1")
            attn_head(1, 0, att_hi[1], att_lo[1])
            mk_u(ps_att, "patt", 1, vl_sb)
            attn_head(1, 1, att_hi[1], att_lo[1])
            mk_r34()

            # ---- mm2 interleaved with the remaining slab-1 heads ----
            with tc.tile_pool(name="ps_o", bufs=4, space="PSUM") as ps_o:
                attn_head(1, 2, att_hi[1], att_lo[1])
                mm2_quarter(ps_o, 0, range(0, 4))
                attn_head(1, 3, att_hi[1], att_lo[1])
                mm2_quarter(ps_o, 0, range(4, 8))
                attn_head(1, 4, att_hi[1], att_lo[1])
                mm2_quarter(ps_o, 1, range(0, 4))
                attn_head(1, 5, att_hi[1], att_lo[1])
                mm2_quarter(ps_o, 1, range(4, 8))
                mm2_quarter(ps_o, 2, range(0, 8))
                mm2_quarter(ps_o, 3, range(0, 8))

    nc.compile()
    return nc


class _Runner:
    """Builds the Bass program once and caches a jitted shard_map executable
    (one batch element per NeuronCore)."""

    IN_ORDER = ["x8", "dx8", "w8i", "dw8i", "w8o", "dw8o", "a_tab", "w34", "wfl"]

    def __init__(self):
        import jax
        from jax.sharding import Mesh, PartitionSpec
        from jax.experimental.shard_map import shard_map

        self.jax = jax
        _b2j.install_neuronx_cc_hook()
        nc = _build_program()
        self.nc = nc
        self.a_tab_np, self.wfl_np = _host_tables()

        partition_name = (
            nc.partition_id_tensor.name if nc.partition_id_tensor else None
        )
        in_names = []
        out_names = []
        out_avals = []
        for alloc in nc.m.functions[0].allocations:
            if not isinstance(alloc, mybir.MemoryLocationSet):
                continue
            name = alloc.memorylocations[0].name
            if alloc.kind == "ExternalInput":
                if name != partition_name:
                    in_names.append(name)
            elif alloc.kind == "ExternalOutput":
                out_names.append(name)
                out_avals.append(
                    jax.core.ShapedArray(
                        tuple(alloc.tensor_shape), mybir.dt.np(alloc.dtype)
                    )
                )
        assert sorted(in_names) == sorted(self.IN_ORDER), in_names
        self.in_names = in_names
        self.out_names = out_names
        self.out_avals = out_avals
        n_params = len(in_names)
        n_outs = len(out_names)
        all_names = tuple(in_names) + tuple(out_names)
        if partition_name is not None:
            all_names = all_names + (partition_name,)

        def _body(*args):
            operands = list(args)
            if partition_name is not None:
                operands.append(_b2j.partition_id_tensor())
            outs = _b2j._bass_exec_p.bind(
                *operands,
                out_avals=tuple(out_avals),
                in_names=all_names,
                out_names=tuple(out_names),
                lowering_input_output_aliases=(),
                sim_require_finite=True,
                sim_require_nnan=True,
                nc=nc,
            )
            return tuple(outs)

        devices = jax.devices()[:B]
        assert len(devices) == B
        self.mesh = Mesh(np.asarray(devices), ("core",))
        in_specs = (PartitionSpec("core"),) * (n_params + n_outs)
        out_specs = (PartitionSpec("core"),) * n_outs
        self.sharded = jax.jit(
            shard_map(
                _body,
                mesh=self.mesh,
                in_specs=in_specs,
                out_specs=out_specs,
                check_rep=False,
            ),
            donate_argnums=tuple(range(n_params, n_params + n_outs)),
            keep_unused=True,
        )

    def _concat_static(self, statics):
        jax = self.jax
        out = {}
        for name, arr in statics.items():
            big = np.concatenate([arr] * B, axis=0)
            out[name] = jax.device_put(big)
        return out

    def run_device(self, dev_args):
        jnp = self.jax.numpy
        zeros = [
            jnp.zeros((B * av.shape[0], *av.shape[1:]), av.dtype)
            for av in self.out_avals
        ]
        return self.sharded(*dev_args, *zeros)

    def prepare_inputs(self, x, W_in, W_out):
        # ---- x: per batch, 2-level e4m3 at scale SX, DoubleRow layout ----
        xs = x.reshape(B, L, E) * np.float32(SX)
        x8 = xs.astype(NPF8)
        dx8 = (xs - x8.astype(np.float32)).astype(NPF8)

        def dr_x(a8):  # [B, L, E] fp8 -> [B*128, NT*1024]
            t = a8.reshape(B, NT, 128, KT2, 2, 128)   # b, i, l, kt2, s, p
            t = t.transpose(0, 5, 1, 3, 4, 2)         # b, p, i, kt2, s, l
            return np.ascontiguousarray(t).reshape(B * 128, NT * 1024)

        # ---- W_in.T: 2-level e4m3 at scale SWI, DoubleRow layout ----
        wiT = W_in.T * np.float32(SWI)
        w8 = wiT.astype(NPF8)
        dw8 = (wiT - w8.astype(np.float32)).astype(NPF8)

        def dr_wi(a8):  # [E, E] fp8 -> [128, KT2*2048]
            t = a8.reshape(KT2, 2, 128, E)            # kt2, s, p, m
            t = t.transpose(2, 0, 1, 3)               # p, kt2, s, m
            return np.ascontiguousarray(t).reshape(128, KT2 * 2048)

        # ---- W_out.T banded rows: 2-level e4m3 at scale SWO, pair layout ----
        woT = W_out.T * np.float32(SWO)
        wo8 = woT.astype(NPF8)
        dwo8 = (woT - wo8.astype(np.float32)).astype(NPF8)

        def dr_wo(a8):  # [E, E] fp8 -> [128, NBH*1024]
            t = np.stack([a8[h * 128:(h + 1) * 128, :] for h in BANDED_HEADS])
            t = t.transpose(1, 0, 2)                  # p, bi, e
            return np.ascontiguousarray(t).reshape(128, NBH * E)

        # ---- W_out.T rows for heads 3/4 (bf16, pre-descaled: u34 carries
        # the v-scale 65536, so fold 1/65536 here to make r34 natural) ----
        w34 = (W_out.T[384:640, :] * np.float32(INV1))
        w34 = w34.reshape(2, 128, E).transpose(1, 0, 2)
        w34 = np.ascontiguousarray(w34).reshape(128, 2 * E).astype(NPBF)

        statics = {
            "w8i": dr_wi(w8),
            "dw8i": dr_wi(dw8),
            "w8o": dr_wo(wo8),
            "dw8o": dr_wo(dwo8),
            "a_tab": self.a_tab_np,
            "w34": w34,
            "wfl": self.wfl_np,
        }
        dev = self._concat_static(statics)
        dev["x8"] = self.jax.device_put(dr_x(x8))
        dev["dx8"] = self.jax.device_put(dr_x(dx8))
        return [dev[name] for name in self.in_names]

    def __call__(self, x, W_in, W_out):
        args = self.prepare_inputs(x, W_in, W_out)
        outs = self.run_device(args)
        outT = np.asarray(outs[self.out_names.index("out")])  # [B*E, L] bf16
        outT = outT.astype(np.float32)
        return np.ascontiguousarray(outT.reshape(B, E, L).transpose(0, 2, 1))


_CACHE = {}


def _get_runner() -> _Runner:
    if "runner" not in _CACHE:
        _CACHE["runner"] = _Runner()
    return _CACHE["runner"]


def kernel(x, W_in, W_out):
    x = np.ascontiguousarray(np.asarray(x, dtype=np.float32))
    W_in = np.ascontiguousarray(np.asarray(W_in, dtype=np.float32))
    W_out = np.ascontiguousarray(np.asarray(W_out, dtype=np.float32))
    assert x.shape == (B, L, E)
    return _get_runner()(x, W_in, W_out)


if __name__ == "__main__":
    rng = np.random.default_rng(0)
    x = rng.standard_normal((B, L, E), dtype=np.float32)
    W_in = rng.standard_normal((E, E), dtype=np.float32) * 0.05
    W_out = rng.standard_normal((E, E), dtype=np.float32) * 0.05
    y = kernel(x, W_in, W_out)
    print("out", y.shape, y.dtype, np.abs(y).mean())
